# revision 19
# baseline (speedup 1.0000x reference)
"""GIN (3-layer) message-passing kernel for Trainium2, 8 NeuronCores.

Strategy (graph-partition data parallel):
  - Graphs are assigned to cores by id: core c owns graphs [c*750, (c+1)*750).
    Nodes are renumbered so each graph occupies a fixed GS-slot stride
    (GS = max graph size, graphs are ~49-51 nodes); slots beyond a graph's
    size duplicate the graph's first node (same in-edges, same degree), so
    the padded slot computes exactly the same z as the duplicated node.
    Segment-max pooling is then a uniform-width reduce, core-local.
  - Edges sharded by destination core.  Host sorts each core's edges (plus one
    self-edge per slot) by local dst slot, groups them into 128-slot blocks,
    and pads each block's edge list to a multiple of 128 ("k-tiles").  The
    k-tile structure is shared across cores (max over cores per block) so the
    SPMD program is identical on all cores.
  - Aggregation: bulk indirect-DMA gathers fetch h[src] rows (bf16, 256B)
    from a shared-HBM table; a per-k-tile one-hot matrix S (vector engine
    iota/is_equal) right-multiplies the gathered tile on the tensor engine,
    accumulating aggT[feat, slot] in PSUM per 128-slot block.
  - MLP runs in transposed space (feat on partitions) in bf16; BatchNorm of
    the previous layer is folded into the next layer's first matmul (w1
    row-scaled by s, plus a rank-1 (w1^T t) x deg correction), so h tables
    stay un-normalized.  BN statistics come free from activation accum_out
    (fp32); a 1KB AllReduce shares them.  Stats are means over the padded
    slot population (duplicates included) - a <0.1% perturbation of BN.
  - The h table lives in the shared DRAM scratchpad; the per-layer AllGather
    writes each core's shard into it directly (no 8x replication traffic).
  - Pooling: per-group on-the-fly segment-max over the bf16 z2 tile (middle
    aligned graphs via rearrange, boundary graphs via partial reduces), then
    the (monotone, gamma>0) BN affine, transpose, concat per-core output.
Host assembles the 8 per-core [750, 384] outputs into the full [6000, 384].
"""

import sys

sys.path.insert(0, "/opt/trn_rl_repo")

import math
from dataclasses import dataclass

import numpy as np

try:
    from ml_dtypes import bfloat16 as np_bf16
except ImportError:  # pragma: no cover
    import jax.numpy as _jnp

    np_bf16 = _jnp.bfloat16

N_GRAPHS = 6000
N_CORES = 8
IN_DIM = 77
DIM = 128
EPS = 1e-5
CALL_KT = 1  # k-tiles per indirect gather call
GRP_BLKS = 4  # 128-slot blocks per MLP group (=512 cols)


@dataclass
class HostData:
    gs: int  # padded graph stride (max graph size)
    gpc: int  # graphs per core
    slots: int  # real+dup slots per core (gpc*gs)
    nb: int  # 128-slot blocks per core
    shp: int  # padded slots per core (nb*128)
    kt_total: int
    blk_kt0: np.ndarray  # [nb] first k-tile of each block
    blk_nk: np.ndarray  # [nb] k-tiles per block
    idx_sb: list  # per core [128, KT] int32 gather row ids
    rel_sb: list  # per core [128, KT] f32 dst-in-block (or -1 pad)
    deg2: list  # per core [128, ncolg*GW] bf16 (rank-1 fold layout)
    x_tbl: np.ndarray  # [tbl, 128] bf16

    @property
    def tbl(self):
        return N_CORES * self.shp

    @property
    def ng(self):
        return (self.nb + GRP_BLKS - 1) // GRP_BLKS


def prep_host(x: np.ndarray, edge_index: np.ndarray, batch: np.ndarray) -> HostData:
    C = N_CORES
    N = x.shape[0]
    batch = batch.astype(np.int64)
    sizes = np.bincount(batch, minlength=N_GRAPHS)
    assert sizes.min() >= 1
    starts = np.concatenate([[0], np.cumsum(sizes)[:-1]])
    GS = int(sizes.max())
    GPC = N_GRAPHS // C
    SLOTS = GPC * GS
    NB = (SLOTS + 127) // 128
    SHP = NB * 128
    TBL = C * SHP

    # node -> (core, local slot, global table row)
    g_of = batch
    pos = np.arange(N, dtype=np.int64) - starts[g_of]
    core_of = g_of // GPC
    slot_loc = (g_of - core_of * GPC) * GS + pos
    row_of = (core_of * SHP + slot_loc).astype(np.int64)

    src = edge_index[0].astype(np.int64)
    dst = edge_index[1].astype(np.int64)

    # destination-side entries: (core, dslot, src_row)
    e_core = [core_of[dst], core_of]
    e_dslot = [slot_loc[dst], slot_loc]
    e_srow = [row_of[src], row_of]  # real edges + self edges

    # duplicate slots: graph g's pad slots [size_g, GS) copy n0 = starts[g]
    n0_edges = np.where(dst == starts[g_of[dst]])[0]  # edges into any n0
    n0_g = g_of[dst[n0_edges]]
    max_pad = GS - int(sizes.min())
    for j in range(max_pad):
        gsel_mask = sizes + j < GS  # graphs needing pad slot at size_g + j
        # in-edges of n0 for selected graphs
        em = gsel_mask[n0_g]
        gg = n0_g[em]
        pc = gg // GPC
        ps = (gg - pc * GPC) * GS + sizes[gg] + j
        e_core.append(pc)
        e_dslot.append(ps)
        e_srow.append(row_of[src[n0_edges[em]]])
        # the duplicated node's self term: edge from n0's row
        gsel = np.where(gsel_mask)[0]
        pc2 = gsel // GPC
        e_core.append(pc2)
        e_dslot.append((gsel - pc2 * GPC) * GS + sizes[gsel] + j)
        e_srow.append(row_of[starts[gsel]])

    e_core = np.concatenate(e_core)
    e_dslot = np.concatenate(e_dslot)
    e_srow = np.concatenate(e_srow)

    per_core = []
    cnts = np.zeros((C, NB), dtype=np.int64)
    for c in range(C):
        m = e_core == c
        dl_c, sr_c = e_dslot[m], e_srow[m]
        order = np.argsort(dl_c, kind="stable")
        dl_c, sr_c = dl_c[order], sr_c[order]
        blk = dl_c >> 7
        cnts[c] = np.bincount(blk, minlength=NB)
        per_core.append((sr_c, dl_c, blk))

    blk_nk = (cnts.max(axis=0) + 127) // 128  # shared k-tile structure
    blk_nk = np.maximum(blk_nk, 1)
    blk_kt0 = np.concatenate([[0], np.cumsum(blk_nk)[:-1]])
    KT = int(blk_nk.sum())
    k_pad = KT * 128

    NG = (NB + GRP_BLKS - 1) // GRP_BLKS
    GW = GRP_BLKS * 128
    ncolg = (NG + 1) // 2

    idx_sb, rel_sb, deg2 = [], [], []
    for c in range(C):
        sr_c, dl_c, blk = per_core[c]
        bstart = np.concatenate([[0], np.cumsum(cnts[c])[:-1]])
        p = np.arange(len(sr_c)) - bstart[blk]
        slot = blk_kt0[blk] * 128 + p
        idx_arr = np.zeros(k_pad, dtype=np.int32)
        rel_arr = np.full(k_pad, -1.0, dtype=np.float32)
        idx_arr[slot] = sr_c.astype(np.int32)
        rel_arr[slot] = (dl_c & 127).astype(np.float32)
        idx_sb.append(np.ascontiguousarray(idx_arr.reshape(KT, 128).T))
        rel_sb.append(np.ascontiguousarray(rel_arr.reshape(KT, 128).T))

        # per-slot degree (= in-edges incl self) for the rank-1 BN-fold matmul
        deg_p = np.bincount(dl_c, minlength=SHP).astype(np.float32)
        d2 = np.zeros((128, ncolg * GW), dtype=np.float32)
        for g in range(NG):
            seg = deg_p[g * GW : (g + 1) * GW]
            d2[(g % 2) * 64, (g // 2) * GW : (g // 2) * GW + len(seg)] = seg
        deg2.append(d2.astype(np_bf16))

    x_tbl = np.zeros((TBL, 128), dtype=np_bf16)
    x_tbl[row_of, :IN_DIM] = x.astype(np_bf16)

    return HostData(GS, GPC, SLOTS, NB, SHP, KT, blk_kt0, blk_nk, idx_sb, rel_sb, deg2, x_tbl)


def build_program(hd: HostData):
    """Returns (nc, input_names)."""
    import concourse.bass as bass
    import concourse.mybir as mybir
    import concourse.tile as tile
    from concourse import bacc
    from concourse.masks import make_identity

    dt = mybir.dt
    Alu = mybir.AluOpType
    Act = mybir.ActivationFunctionType

    C, D = N_CORES, DIM
    NB, SHP, TBL, NG, KT = hd.nb, hd.shp, hd.tbl, hd.ng, hd.kt_total
    GW = GRP_BLKS * 128
    GS, GPC, SLOTS = hd.gs, hd.gpc, hd.slots
    ncolg = (NG + 1) // 2
    inv_n = 1.0 / (C * SLOTS)

    nc = bacc.Bacc(
        "TRN2", target_bir_lowering=False, debug=False, num_devices=C
    )

    def din(name, shape, dtp=dt.float32):
        return nc.dram_tensor(name, list(shape), dtp, kind="ExternalInput").ap()

    x_tbl_d = din("x_tbl", (TBL, D), dt.bfloat16)
    idx_d = din("idx", (128, KT), dt.int32)
    rel_d = din("rel", (128, KT))
    deg2_d = din("deg2", (128, ncolg * GW), dt.bfloat16)
    iota_d = din("iota", (128, 128), dt.bfloat16)
    w1_d = [din(f"w1_{l}", (D, D)) for l in range(3)]
    w2_d = [din(f"w2_{l}", (D, D)) for l in range(3)]
    b1_d = [din(f"b1_{l}", (D, 1)) for l in range(3)]
    b2_d = [din(f"b2_{l}", (D, 1)) for l in range(3)]
    gb_d = din("gb", (D, 6))  # cols: g0 b0 g1 b1 g2 b2
    out_d = nc.dram_tensor(
        "pooled", [GPC, 3 * D], dt.float32, kind="ExternalOutput"
    ).ap()

    input_names = (
        ["x_tbl", "idx", "rel", "deg2", "iota"]
        + [f"w1_{l}" for l in range(3)]
        + [f"w2_{l}" for l in range(3)]
        + [f"b1_{l}" for l in range(3)]
        + [f"b2_{l}" for l in range(3)]
        + ["gb"]
    )

    n_pool_chunks = (GPC + 127) // 128
    last_chunk_rows = GPC - (n_pool_chunks - 1) * 128

    with tile.TileContext(nc) as tc:
        with (
            tc.tile_pool(name="const", bufs=1) as cpool,
            tc.tile_pool(name="ebuf", bufs=4) as epool,
            tc.tile_pool(name="spool", bufs=4) as spool,
            tc.tile_pool(name="zin", bufs=2) as zinpool,
            tc.tile_pool(name="zmid", bufs=2) as zmidpool,
            tc.tile_pool(name="rm", bufs=3) as rmpool,
            tc.tile_pool(name="stat", bufs=1) as statpool,
            tc.tile_pool(name="agg_ps", bufs=2, space="PSUM") as aggpool,
            tc.tile_pool(name="m1_ps", bufs=2, space="PSUM") as m1pool,
            tc.tile_pool(name="m2_ps", bufs=2, space="PSUM") as m2pool,
            tc.tile_pool(name="tr_ps", bufs=2, space="PSUM") as trpool,
            tc.tile_pool(name="dram", bufs=1, space="DRAM") as dpool,
        ):
            # ---- DRAM intermediates ----
            h_tbls = [
                dpool.tile(
                    [TBL, D], dt.bfloat16, name=f"h_tbl{l}",
                    addr_space="Shared",
                )
                for l in range(2)
            ]
            z_rm = dpool.tile([SHP, D], dt.bfloat16, name="z_rm")
            st_in = [
                dpool.tile([D, 2], dt.float32, name=f"st_in{l}") for l in range(3)
            ]
            st_out = [
                dpool.tile([D, 2], dt.float32, name=f"st_out{l}")
                for l in range(3)
            ]

            # ---- constants to SBUF ----
            def load(shape, src_ap, dtp=dt.float32, name=None):
                t = cpool.tile(list(shape), dtp, name=name)
                nc.sync.dma_start(out=t[:], in_=src_ap)
                return t

            idx_sb = load((128, KT), idx_d[:], dt.int32, name="idx_sb")
            rel_sb = load((128, KT), rel_d[:], name="rel_sb")
            deg2_sb = load(
                (128, ncolg * GW), deg2_d[:], dt.bfloat16, name="deg2_sb"
            )
            iota_sb = load((128, 128), iota_d[:], dt.bfloat16, name="iota_sb")
            w1_sb = [load((D, D), w1_d[l][:], name=f"w1sb{l}") for l in range(3)]
            w2_sb = [load((D, D), w2_d[l][:], name=f"w2sb{l}") for l in range(3)]
            b1_sb = [load((D, 1), b1_d[l][:], name=f"b1sb{l}") for l in range(3)]
            b2_sb = [load((D, 1), b2_d[l][:], name=f"b2sb{l}") for l in range(3)]
            gb_sb = load((D, 6), gb_d[:], name="gb_sb")
            w1r0 = cpool.tile([D, D], dt.float32, name="w1r0")
            nc.any.tensor_copy(out=w1r0[:], in_=w1_sb[0][:])
            w2r = []
            for l in range(3):
                t = cpool.tile([D, D], dt.float32, name=f"w2r{l}")
                nc.any.tensor_copy(out=t[:], in_=w2_sb[l][:])
                w2r.append(t)
            ident = cpool.tile([128, 128], dt.bfloat16, name="ident")
            make_identity(nc, ident[:])
            ident32 = cpool.tile([128, 128], dt.float32, name="ident32")
            make_identity(nc, ident32[:])

            # persistent small tiles
            s_all = cpool.tile([D, 3], dt.float32, name="s_all")
            t_all = cpool.tile([D, 3], dt.float32, name="t_all")
            w1s_sb = [
                cpool.tile([D, D], dt.float32, name=f"w1s{l}") for l in (1, 2)
            ]
            u_sb = [cpool.tile([1, D], dt.float32, name=f"u{l}") for l in (1, 2)]
            ub_sb = [
                cpool.tile([D, D], dt.bfloat16, name=f"ub{l}") for l in (1, 2)
            ]
            ones_row = cpool.tile([1, D], dt.float32, name="ones_row")
            nc.gpsimd.memset(ones_row[:], 1.0)
            ssum = cpool.tile([128, NG], dt.float32, name="ssum")
            ssq = cpool.tile([128, NG], dt.float32, name="ssq")
            sq_scr = cpool.tile([128, GW], dt.float32, name="sq_scr")
            stat_scr = cpool.tile([128, 8], dt.float32, name="stat_scr")
            pt_all = [
                cpool.tile([128, GPC], dt.float32, name=f"pt{l}")
                for l in range(3)
            ]

            def compute_fold(l):
                """Load layer-l AR'd stats; fill s_all/t_all col l and (for
                l<2) w1s_sb/u_sb of layer l+1."""
                st = statpool.tile([D, 2], dt.float32, name="st_ld")
                nc.sync.dma_start(out=st[:], in_=st_out[l][:])
                mu = stat_scr[:, 0:1]
                msq = stat_scr[:, 1:2]
                var = stat_scr[:, 2:3]
                rstd = stat_scr[:, 3:4]
                smu = stat_scr[:, 4:5]
                nc.vector.tensor_scalar_mul(mu, st[:, 0:1], inv_n)
                nc.vector.tensor_scalar_mul(msq, st[:, 1:2], inv_n)
                nc.vector.tensor_tensor(out=var, in0=mu, in1=mu, op=Alu.mult)
                nc.vector.tensor_tensor(
                    out=var, in0=msq, in1=var, op=Alu.subtract
                )
                veps = stat_scr[:, 6:7]
                nc.vector.tensor_scalar_add(veps, var, EPS)
                std = stat_scr[:, 5:6]
                nc.scalar.activation(std, veps, Act.Sqrt)
                nc.vector.reciprocal(rstd, std)
                scol = s_all[:, l : l + 1]
                tcol = t_all[:, l : l + 1]
                nc.vector.tensor_tensor(
                    out=scol, in0=gb_sb[:, 2 * l : 2 * l + 1], in1=rstd,
                    op=Alu.mult,
                )
                nc.vector.tensor_tensor(out=smu, in0=scol, in1=mu, op=Alu.mult)
                nc.vector.tensor_tensor(
                    out=tcol, in0=gb_sb[:, 2 * l + 1 : 2 * l + 2], in1=smu,
                    op=Alu.subtract,
                )
                if l < 2:
                    ln = l + 1
                    nc.vector.tensor_scalar(
                        out=w1s_sb[ln - 1][:], in0=w1_sb[ln][:], scalar1=scol,
                        scalar2=None, op0=Alu.mult,
                    )
                    ups = trpool.tile([1, D], dt.float32, name="ups", tag="tr")
                    nc.tensor.matmul(
                        ups[:], lhsT=tcol, rhs=w1_sb[ln][:], start=True,
                        stop=True,
                    )
                    nc.any.tensor_copy(out=u_sb[ln - 1][:], in_=ups[:])
                    ubp = trpool.tile([D, D], dt.float32, name="ubp", tag="tr")
                    nc.tensor.matmul(
                        ubp[:], lhsT=ones_row[:], rhs=u_sb[ln - 1][:],
                        start=True, stop=True,
                    )
                    nc.any.tensor_copy(out=ub_sb[ln - 1][:], in_=ubp[:])

            for layer in range(3):
                tbl_ap = x_tbl_d if layer == 0 else h_tbls[layer - 1][:]
                if layer > 0:
                    compute_fold(layer - 1)
                lhs1 = w1r0 if layer == 0 else w1s_sb[layer - 1]
                pt = pt_all[layer]

                ecur = [None]
                ecall = [-1]

                def e_slice(t):
                    call = t // CALL_KT
                    if call != ecall[0]:
                        w = min(CALL_KT, KT - call * CALL_KT)
                        et = epool.tile(
                            [128, CALL_KT * 128], dt.bfloat16, name="ebuf"
                        )
                        nc.gpsimd.indirect_dma_start(
                            out=et[:, : w * 128],
                            out_offset=None,
                            in_=tbl_ap,
                            in_offset=bass.IndirectOffsetOnAxis(
                                ap=idx_sb[
                                    :, call * CALL_KT : call * CALL_KT + w
                                ],
                                axis=0,
                            ),
                        )
                        ecur[0], ecall[0] = et, call
                    p = t - call * CALL_KT
                    return ecur[0][:, p * 128 : (p + 1) * 128]

                for g in range(NG):
                    blo = g * GRP_BLKS
                    bhi = min(blo + GRP_BLKS, NB)
                    W = (bhi - blo) * 128
                    zin = zinpool.tile([128, GW], dt.float32, name="zin")
                    for b in range(blo, bhi):
                        agg = aggpool.tile([128, 128], dt.float32, name="agg")
                        nk = int(hd.blk_nk[b])
                        t0 = int(hd.blk_kt0[b])
                        for j in range(nk):
                            esl = e_slice(t0 + j)
                            s_t = spool.tile(
                                [128, 128], dt.bfloat16, name="s_t"
                            )
                            nc.vector.tensor_scalar(
                                out=s_t[:], in0=iota_sb[:],
                                scalar1=rel_sb[:, t0 + j : t0 + j + 1],
                                scalar2=None, op0=Alu.is_equal,
                            )
                            nc.tensor.matmul(
                                agg[:], lhsT=esl, rhs=s_t[:],
                                start=(j == 0), stop=(j == nk - 1),
                            )
                        co = (b - blo) * 128
                        nc.any.tensor_copy(
                            out=zin[:, co : co + 128], in_=agg[:]
                        )
                    # ---- MLP on the group (transposed space, fp32r) ----
                    m1 = m1pool.tile([128, GW], dt.float32, name="m1")
                    nc.tensor.matmul(
                        m1[:, :W], lhsT=lhs1[:], rhs=zin[:, :W],
                        start=True, stop=(layer == 0),
                    )
                    if layer > 0:
                        dp = (g % 2) * 64
                        dc = (g // 2) * GW
                        nc.tensor.matmul(
                            m1[:, :W], lhsT=ub_sb[layer - 1][dp : dp + 1, :],
                            rhs=deg2_sb[dp : dp + 1, dc : dc + W],
                            start=False, stop=True,
                        )
                    z1 = zmidpool.tile([128, GW], dt.float32, name="z1")
                    nc.scalar.activation(
                        z1[:, :W], m1[:, :W], Act.Relu, bias=b1_sb[layer][:]
                    )
                    m2 = m2pool.tile([128, GW], dt.float32, name="m2")
                    nc.tensor.matmul(
                        m2[:, :W], lhsT=w2r[layer][:], rhs=z1[:, :W],
                        start=True, stop=True,
                    )
                    z2 = zmidpool.tile([128, GW], dt.bfloat16, name="z2")
                    c0 = g * GW
                    wr = min(W, max(0, SLOTS - c0))  # stat cols (real+dup)
                    if wr > 0:
                        nc.scalar.activation(
                            z2[:, :wr], m2[:, :wr], Act.Relu,
                            bias=b2_sb[layer][:], accum_out=ssum[:, g : g + 1],
                        )
                    if wr < W:
                        nc.scalar.activation(
                            z2[:, wr:W], m2[:, wr:W], Act.Relu,
                            bias=b2_sb[layer][:],
                        )
                    if wr > 0:
                        nc.scalar.activation(
                            sq_scr[:, :wr], z2[:, :wr], Act.Square,
                            accum_out=ssq[:, g : g + 1],
                        )
                    # ---- on-the-fly pooling (raw m2, fp32; relu+b2 at end) ----
                    pc1 = min(c0 + W, SLOTS)
                    if c0 < pc1:
                        gfirst = (c0 + GS - 1) // GS
                        a = gfirst * GS - c0
                        gend = pc1 // GS
                        nfull = gend - gfirst
                        if nfull > 0:
                            nc.vector.tensor_reduce(
                                out=pt[:, gfirst:gend],
                                in_=m2[:, a : a + nfull * GS].rearrange(
                                    "p (g s) -> p g s", s=GS
                                ),
                                axis=mybir.AxisListType.X, op=Alu.max,
                            )
                        if a > 0:  # left partial graph gfirst-1
                            la = min(a, pc1 - c0)
                            tmpm = stat_scr[:, 7:8]
                            nc.vector.tensor_reduce(
                                out=tmpm, in_=m2[:, 0:la],
                                axis=mybir.AxisListType.X, op=Alu.max,
                            )
                            gl = gfirst - 1
                            nc.vector.tensor_tensor(
                                out=pt[:, gl : gl + 1],
                                in0=pt[:, gl : gl + 1], in1=tmpm, op=Alu.max,
                            )
                        r0 = a + max(0, gend - gfirst) * GS
                        if gend >= gfirst and c0 + r0 < pc1:
                            # right partial graph gend (first touch)
                            nc.vector.tensor_reduce(
                                out=pt[:, gend : gend + 1],
                                in_=m2[:, r0 : pc1 - c0],
                                axis=mybir.AxisListType.X, op=Alu.max,
                            )
                    # ---- transpose to node-major for the h table ----
                    if layer < 2:
                        for i in range(W // 128):
                            trp = trpool.tile(
                                [128, 128], dt.bfloat16, name="trp", tag="tr"
                            )
                            nc.tensor.transpose(
                                trp[:], z2[:, i * 128 : (i + 1) * 128],
                                ident[:],
                            )
                            rm = rmpool.tile([128, 128], dt.bfloat16, name="rm")
                            nc.any.tensor_copy(out=rm[:], in_=trp[:])
                            r0 = g * GW + i * 128
                            nc.sync.dma_start(
                                out=z_rm[r0 : r0 + 128, :], in_=rm[:]
                            )

                # ---- stats reduce + AllReduce ----
                sp = statpool.tile([D, 2], dt.float32, name="sp")
                nc.vector.tensor_reduce(
                    out=sp[:, 0:1], in_=ssum[:, :NG],
                    axis=mybir.AxisListType.X, op=Alu.add,
                )
                nc.vector.tensor_reduce(
                    out=sp[:, 1:2], in_=ssq[:, :NG],
                    axis=mybir.AxisListType.X, op=Alu.add,
                )
                nc.sync.dma_start(out=st_in[layer][:], in_=sp[:])
                nc.gpsimd.collective_compute(
                    "AllReduce", mybir.AluOpType.add,
                    replica_groups=[list(range(C))],
                    ins=[st_in[layer].opt()], outs=[st_out[layer].opt()],
                )
                if layer < 2:
                    nc.gpsimd.collective_compute(
                        "AllGather", mybir.AluOpType.bypass,
                        replica_groups=[list(range(C))],
                        ins=[z_rm.opt()], outs=[h_tbls[layer].opt()],
                    )

            # ---- output: affine + transpose + store ----
            compute_fold(2)
            out_big = cpool.tile(
                [128, n_pool_chunks * 3 * D], dt.float32, name="out_big"
            )
            with tc.tile_pool(name="poolt", bufs=2) as ptpool:
                for l in range(3):
                    # pooled z2 = relu(max(m2) + b2); then BN affine
                    pre = ptpool.tile([128, GPC], dt.float32, name="pre")
                    nc.scalar.activation(
                        pre[:], pt_all[l][:], Act.Relu, bias=b2_sb[l][:]
                    )
                    pta = ptpool.tile([128, GPC], dt.float32, name="pta")
                    nc.vector.tensor_scalar(
                        out=pta[:], in0=pre[:],
                        scalar1=s_all[:, l : l + 1],
                        scalar2=t_all[:, l : l + 1], op0=Alu.mult, op1=Alu.add,
                    )
                    for ch in range(n_pool_chunks):
                        rows = (
                            128 if ch < n_pool_chunks - 1 else last_chunk_rows
                        )
                        trp = trpool.tile(
                            [128, 128], dt.float32, name="trpo", tag="tr"
                        )
                        nc.tensor.transpose(
                            trp[:rows, :],
                            pta[:, ch * 128 : ch * 128 + rows], ident32[:],
                        )
                        nc.any.tensor_copy(
                            out=out_big[
                                :rows, ch * 3 * D + l * D : ch * 3 * D
                                + (l + 1) * D
                            ],
                            in_=trp[:rows, :],
                        )
            for ch in range(n_pool_chunks):
                rows = 128 if ch < n_pool_chunks - 1 else last_chunk_rows
                nc.sync.dma_start(
                    out=out_d[ch * 128 : ch * 128 + rows, :],
                    in_=out_big[:rows, ch * 3 * D : (ch + 1) * 3 * D],
                )

    nc.compile()
    return nc, input_names


def make_in_maps(hd: HostData, inputs: dict, input_names):
    iota = np.tile(np.arange(128, dtype=np.float32), (128, 1)).astype(np_bf16)
    gb = np.zeros((DIM, 6), dtype=np.float32)
    for l in range(3):
        gb[:, 2 * l] = inputs["gamma"][l]
        gb[:, 2 * l + 1] = inputs["beta"][l]
    shared = {
        "x_tbl": hd.x_tbl,
        "iota": np.ascontiguousarray(iota),
        "gb": gb,
    }
    for l in range(3):
        w = np.zeros((DIM, DIM), dtype=np.float32)
        wl = inputs[f"w1_{l}"]
        w[: wl.shape[0], :] = wl
        shared[f"w1_{l}"] = w
        shared[f"w2_{l}"] = np.ascontiguousarray(
            inputs[f"w2_{l}"].astype(np.float32)
        )
        shared[f"b1_{l}"] = inputs[f"b1_{l}"].astype(np.float32).reshape(-1, 1)
        shared[f"b2_{l}"] = inputs[f"b2_{l}"].astype(np.float32).reshape(-1, 1)
    in_maps = []
    for c in range(N_CORES):
        m = dict(shared)
        m["idx"] = hd.idx_sb[c]
        m["rel"] = hd.rel_sb[c]
        m["deg2"] = hd.deg2[c]
        assert set(m.keys()) == set(input_names)
        in_maps.append(m)
    return in_maps


def _run_sharded_timed(nc, in_maps, n_cores, iters=10, warmup=2):
    """Execute the compiled Bass module via PJRT with device-resident inputs,
    timing `iters` back-to-back dispatches (excludes input upload/compile)."""
    import time

    import jax
    from jax.sharding import Mesh, NamedSharding, PartitionSpec
    from jax.experimental.shard_map import shard_map

    import concourse.mybir as mybir
    from concourse import bass2jax

    bass2jax.install_neuronx_cc_hook()
    partition_name = (
        nc.partition_id_tensor.name if nc.partition_id_tensor else None
    )
    in_names, out_names, out_avals, zero_outs = [], [], [], []
    for alloc in nc.m.functions[0].allocations:
        if not isinstance(alloc, mybir.MemoryLocationSet):
            continue
        name = alloc.memorylocations[0].name
        if alloc.kind == "ExternalInput":
            if name != partition_name:
                in_names.append(name)
        elif alloc.kind == "ExternalOutput":
            out_names.append(name)
            shape = tuple(alloc.tensor_shape)
            dtp = mybir.dt.np(alloc.dtype)
            out_avals.append(jax.core.ShapedArray(shape, dtp))
            zero_outs.append(np.zeros(shape, dtp))
    n_params, n_outs = len(in_names), len(out_avals)
    in_names.extend(out_names)
    if partition_name is not None:
        in_names.append(partition_name)
    donate = tuple(range(n_params, n_params + n_outs))

    def _body(*args):
        operands = list(args)
        if partition_name is not None:
            operands.append(bass2jax.partition_id_tensor())
        outs = bass2jax._bass_exec_p.bind(
            *operands,
            out_avals=tuple(out_avals),
            in_names=tuple(in_names),
            out_names=tuple(out_names),
            lowering_input_output_aliases=(),
            sim_require_finite=True,
            sim_require_nnan=True,
            nc=nc,
        )
        return tuple(outs)

    devices = jax.devices()[:n_cores]
    mesh = Mesh(np.asarray(devices), ("core",))
    pspec = PartitionSpec("core")
    in_specs = (pspec,) * (n_params + n_outs)
    sharded = jax.jit(
        shard_map(
            _body, mesh=mesh, in_specs=in_specs,
            out_specs=(pspec,) * len(out_names), check_rep=False,
        ),
        donate_argnums=donate, keep_unused=True,
    )
    shd = NamedSharding(mesh, pspec)
    per_core = [
        [np.asarray(m[name]) for name in in_names[:n_params]] for m in in_maps
    ]
    dev_in = [
        jax.device_put(
            np.concatenate([per_core[c][i] for c in range(n_cores)], axis=0),
            shd,
        )
        for i in range(n_params)
    ]
    n_calls = warmup + (iters if iters else 0)
    zsets = [
        [
            jax.device_put(
                np.zeros((n_cores * z.shape[0], *z.shape[1:]), z.dtype), shd
            )
            for z in zero_outs
        ]
        for _ in range(max(n_calls, 1))
    ]
    outs = None
    for i in range(warmup):
        outs = sharded(*dev_in, *zsets[i])
        jax.block_until_ready(outs)
    dt = None
    if iters:
        t0 = time.perf_counter()
        ress = [sharded(*dev_in, *zsets[warmup + i]) for i in range(iters)]
        jax.block_until_ready(ress)
        dt = (time.perf_counter() - t0) / iters
        outs = ress[-1]
    if outs is None:
        outs = sharded(*dev_in, *zsets[0])
    results = [
        {
            name: np.asarray(outs[i]).reshape(n_cores, *out_avals[i].shape)[c]
            for i, name in enumerate(out_names)
        }
        for c in range(n_cores)
    ]
    return results, dt


def run(inputs: dict, timed: bool = False):
    x = np.asarray(inputs["x"])
    ei = np.asarray(inputs["edge_index"])
    batch = np.asarray(inputs["batch"])
    hd = prep_host(x, ei, batch)
    nc, input_names = build_program(hd)
    in_maps = make_in_maps(hd, inputs, input_names)
    results, dt = _run_sharded_timed(
        nc, in_maps, N_CORES,
        iters=(10 if timed else 0), warmup=(2 if timed else 1),
    )
    outs = [results[c]["pooled"] for c in range(N_CORES)]
    full = np.concatenate(outs, axis=0).astype(np.float32)
    return full, dt


def kernel(**inputs) -> np.ndarray:
    out, _ = run(inputs, timed=False)
    return out


# revision 26
# speedup vs baseline: 1.1230x; 1.1230x over previous
"""GIN (3-layer) message-passing kernel for Trainium2, 8 NeuronCores.

Strategy (graph-partition data parallel):
  - Graphs are assigned to cores by id: core c owns graphs [c*750, (c+1)*750).
    Nodes are renumbered so each graph occupies a fixed GS-slot stride
    (GS = max graph size, graphs are ~49-51 nodes); slots beyond a graph's
    size duplicate the graph's first node (same in-edges, same degree), so
    the padded slot computes exactly the same z as the duplicated node.
    Segment-max pooling is then a uniform-width reduce, core-local.
  - Edges sharded by destination core.  Host sorts each core's edges (plus one
    self-edge per slot) by local dst slot, groups them into 128-slot blocks,
    and pads each block's edge list to a multiple of 128 ("k-tiles").  The
    k-tile structure is shared across cores (max over cores per block) so the
    SPMD program is identical on all cores.
  - Aggregation: bulk indirect-DMA gathers fetch h[src] rows (bf16, 256B)
    from a shared-HBM table; a per-k-tile one-hot matrix S (vector engine
    iota/is_equal) right-multiplies the gathered tile on the tensor engine,
    accumulating aggT[feat, slot] in PSUM per 128-slot block.
  - MLP runs in transposed space (feat on partitions) in bf16; BatchNorm of
    the previous layer is folded into the next layer's first matmul (w1
    row-scaled by s, plus a rank-1 (w1^T t) x deg correction), so h tables
    stay un-normalized.  BN statistics come free from activation accum_out
    (fp32); a 1KB AllReduce shares them.  Stats are means over the padded
    slot population (duplicates included) - a <0.1% perturbation of BN.
  - The h table lives in the shared DRAM scratchpad; the per-layer AllGather
    writes each core's shard into it directly (no 8x replication traffic).
  - Pooling: per-group on-the-fly segment-max over the bf16 z2 tile (middle
    aligned graphs via rearrange, boundary graphs via partial reduces), then
    the (monotone, gamma>0) BN affine, transpose, concat per-core output.
Host assembles the 8 per-core [750, 384] outputs into the full [6000, 384].
"""

import sys

sys.path.insert(0, "/opt/trn_rl_repo")

import math
from dataclasses import dataclass

import numpy as np

try:
    from ml_dtypes import bfloat16 as np_bf16
except ImportError:  # pragma: no cover
    import jax.numpy as _jnp

    np_bf16 = _jnp.bfloat16

N_GRAPHS = 6000
N_CORES = 8
IN_DIM = 77
DIM = 128
EPS = 1e-5
CALL_KT = 1  # k-tiles per indirect gather call
GRP_BLKS = 4  # 128-slot blocks per MLP group (=512 cols)
N_CHUNKS = 4  # h-table chunks for overlapped AllGathers


@dataclass
class HostData:
    gs: int  # padded graph stride (max graph size)
    gpc: int  # graphs per core
    slots: int  # real+dup slots per core (gpc*gs)
    nb: int  # 128-slot blocks per core
    shp: int  # padded slots per core (nb*128)
    kt_total: int
    blk_kt0: np.ndarray  # [nb] first k-tile of each block
    blk_nk: np.ndarray  # [nb] k-tiles per block
    idx_sb: list  # per core [128, KT] int32 gather row ids
    rel_sb: list  # per core [128, KT] f32 dst-in-block (or -1 pad)
    deg2: list  # per core [128, ncolg*GW] bf16 (rank-1 fold layout)
    x_tbl: np.ndarray  # [tbl, 128] bf16
    cb: np.ndarray  # [N_CHUNKS+1] chunk boundaries in blocks

    @property
    def tbl(self):
        return N_CORES * self.shp

    @property
    def ng(self):
        return (self.nb + GRP_BLKS - 1) // GRP_BLKS


def prep_host(x: np.ndarray, edge_index: np.ndarray, batch: np.ndarray) -> HostData:
    C = N_CORES
    N = x.shape[0]
    batch = batch.astype(np.int64)
    sizes = np.bincount(batch, minlength=N_GRAPHS)
    assert sizes.min() >= 1
    starts = np.concatenate([[0], np.cumsum(sizes)[:-1]])
    GS = int(sizes.max())
    GPC = N_GRAPHS // C
    SLOTS = GPC * GS
    NB = (SLOTS + 127) // 128
    SHP = NB * 128
    TBL = C * SHP

    # chunked table layout: chunk k holds blocks [cb[k], cb[k+1]) of all cores
    nb_ch = [NB // N_CHUNKS + (1 if i < NB % N_CHUNKS else 0) for i in range(N_CHUNKS)]
    cb = np.concatenate([[0], np.cumsum(nb_ch)]).astype(np.int64)
    shq = [int(n) * 128 for n in nb_ch]
    chunk_base = np.concatenate([[0], np.cumsum([C * s for s in shq])]).astype(np.int64)

    def slot_to_row(core, slot):
        blk = slot >> 7
        k = np.searchsorted(cb[1:], blk, side="right")
        return chunk_base[k] + core * np.asarray(shq)[k] + (slot - cb[k] * 128)

    # node -> (core, local slot, global table row)
    g_of = batch
    pos = np.arange(N, dtype=np.int64) - starts[g_of]
    core_of = g_of // GPC
    slot_loc = (g_of - core_of * GPC) * GS + pos
    row_of = slot_to_row(core_of, slot_loc).astype(np.int64)

    src = edge_index[0].astype(np.int64)
    dst = edge_index[1].astype(np.int64)

    # destination-side entries: (core, dslot, src_row)
    e_core = [core_of[dst], core_of]
    e_dslot = [slot_loc[dst], slot_loc]
    e_srow = [row_of[src], row_of]  # real edges + self edges

    # duplicate slots: graph g's pad slots [size_g, GS) copy n0 = starts[g]
    n0_edges = np.where(dst == starts[g_of[dst]])[0]  # edges into any n0
    n0_g = g_of[dst[n0_edges]]
    max_pad = GS - int(sizes.min())
    for j in range(max_pad):
        gsel_mask = sizes + j < GS  # graphs needing pad slot at size_g + j
        # in-edges of n0 for selected graphs
        em = gsel_mask[n0_g]
        gg = n0_g[em]
        pc = gg // GPC
        ps = (gg - pc * GPC) * GS + sizes[gg] + j
        e_core.append(pc)
        e_dslot.append(ps)
        e_srow.append(row_of[src[n0_edges[em]]])
        # the duplicated node's self term: edge from n0's row
        gsel = np.where(gsel_mask)[0]
        pc2 = gsel // GPC
        e_core.append(pc2)
        e_dslot.append((gsel - pc2 * GPC) * GS + sizes[gsel] + j)
        e_srow.append(row_of[starts[gsel]])

    e_core = np.concatenate(e_core)
    e_dslot = np.concatenate(e_dslot)
    e_srow = np.concatenate(e_srow)

    per_core = []
    cnts = np.zeros((C, NB), dtype=np.int64)
    for c in range(C):
        m = e_core == c
        dl_c, sr_c = e_dslot[m], e_srow[m]
        order = np.argsort(dl_c, kind="stable")
        dl_c, sr_c = dl_c[order], sr_c[order]
        blk = dl_c >> 7
        cnts[c] = np.bincount(blk, minlength=NB)
        per_core.append((sr_c, dl_c, blk))

    blk_nk = (cnts.max(axis=0) + 127) // 128  # shared k-tile structure
    blk_nk = np.maximum(blk_nk, 1)
    blk_kt0 = np.concatenate([[0], np.cumsum(blk_nk)[:-1]])
    KT = int(blk_nk.sum())
    k_pad = KT * 128

    NG = (NB + GRP_BLKS - 1) // GRP_BLKS
    GW = GRP_BLKS * 128
    ncolg = (NG + 1) // 2

    idx_sb, rel_sb, deg2 = [], [], []
    for c in range(C):
        sr_c, dl_c, blk = per_core[c]
        bstart = np.concatenate([[0], np.cumsum(cnts[c])[:-1]])
        p = np.arange(len(sr_c)) - bstart[blk]
        slot = blk_kt0[blk] * 128 + p
        idx_arr = np.zeros(k_pad, dtype=np.int32)
        rel_arr = np.full(k_pad, -1.0, dtype=np.float32)
        idx_arr[slot] = sr_c.astype(np.int32)
        rel_arr[slot] = (dl_c & 127).astype(np.float32)
        idx_sb.append(np.ascontiguousarray(idx_arr.reshape(KT, 128).T))
        rel_sb.append(np.ascontiguousarray(rel_arr.reshape(KT, 128).T))

        # per-slot degree (= in-edges incl self) for the rank-1 BN-fold matmul
        deg_p = np.bincount(dl_c, minlength=SHP).astype(np.float32)
        d2 = np.zeros((128, ncolg * GW), dtype=np.float32)
        for g in range(NG):
            seg = deg_p[g * GW : (g + 1) * GW]
            d2[(g % 2) * 64, (g // 2) * GW : (g // 2) * GW + len(seg)] = seg
        deg2.append(d2.astype(np_bf16))

    x_tbl = np.zeros((TBL, 128), dtype=np_bf16)
    x_tbl[row_of, :IN_DIM] = x.astype(np_bf16)

    return HostData(GS, GPC, SLOTS, NB, SHP, KT, blk_kt0, blk_nk, idx_sb, rel_sb, deg2, x_tbl, cb)


def build_program(hd: HostData):
    """Returns (nc, input_names)."""
    import concourse.bass as bass
    import concourse.mybir as mybir
    import concourse.tile as tile
    from concourse import bacc
    from concourse.masks import make_identity
    from concourse.tile_rust import add_dep_helper

    dt = mybir.dt
    Alu = mybir.AluOpType
    Act = mybir.ActivationFunctionType

    C, D = N_CORES, DIM
    NB, SHP, TBL, NG, KT = hd.nb, hd.shp, hd.tbl, hd.ng, hd.kt_total
    GW = GRP_BLKS * 128
    GS, GPC, SLOTS = hd.gs, hd.gpc, hd.slots
    ncolg = (NG + 1) // 2
    inv_n = 1.0 / (C * SLOTS)

    nc = bacc.Bacc(
        "TRN2", target_bir_lowering=False, debug=False, num_devices=C
    )

    def din(name, shape, dtp=dt.float32):
        return nc.dram_tensor(name, list(shape), dtp, kind="ExternalInput").ap()

    x_tbl_d = din("x_tbl", (TBL, D), dt.bfloat16)
    idx_d = din("idx", (128, KT), dt.int32)
    rel_d = din("rel", (128, KT))
    deg2_d = din("deg2", (128, ncolg * GW), dt.bfloat16)
    iota_d = din("iota", (128, 128), dt.bfloat16)
    w1_d = [din(f"w1_{l}", (D, D)) for l in range(3)]
    w2_d = [din(f"w2_{l}", (D, D)) for l in range(3)]
    b1_d = [din(f"b1_{l}", (D, 1)) for l in range(3)]
    b2_d = [din(f"b2_{l}", (D, 1)) for l in range(3)]
    gb_d = din("gb", (D, 6))  # cols: g0 b0 g1 b1 g2 b2
    out_d = nc.dram_tensor(
        "pooled", [GPC, 3 * D], dt.float32, kind="ExternalOutput"
    ).ap()

    input_names = (
        ["x_tbl", "idx", "rel", "deg2", "iota"]
        + [f"w1_{l}" for l in range(3)]
        + [f"w2_{l}" for l in range(3)]
        + [f"b1_{l}" for l in range(3)]
        + [f"b2_{l}" for l in range(3)]
        + ["gb"]
    )

    n_pool_chunks = (GPC + 127) // 128
    last_chunk_rows = GPC - (n_pool_chunks - 1) * 128

    with tile.TileContext(nc) as tc:
        with (
            tc.tile_pool(name="const", bufs=1) as cpool,
            tc.tile_pool(name="ebuf", bufs=10) as epool,
            tc.tile_pool(name="spool", bufs=8) as spool,
            tc.tile_pool(name="zin", bufs=2) as zinpool,
            tc.tile_pool(name="zmid", bufs=2) as zmidpool,
            tc.tile_pool(name="rm", bufs=3) as rmpool,
            tc.tile_pool(name="stat", bufs=1) as statpool,
            tc.tile_pool(name="agg_ps", bufs=2, space="PSUM") as aggpool,
            tc.tile_pool(name="m1_ps", bufs=2, space="PSUM") as m1pool,
            tc.tile_pool(name="m2_ps", bufs=2, space="PSUM") as m2pool,
            tc.tile_pool(name="tr_ps", bufs=2, space="PSUM") as trpool,
            tc.tile_pool(name="dram", bufs=1, space="DRAM") as dpool,
        ):
            # ---- DRAM intermediates ----
            cb = [int(v) for v in hd.cb]
            NCH = N_CHUNKS
            shq = [(cb[k + 1] - cb[k]) * 128 for k in range(NCH)]
            # h chunks per layer; consecutive allocation => contiguous region
            h_ch = [
                [
                    dpool.tile(
                        [C * shq[k], D], dt.bfloat16, name=f"h_{l}_{k}",
                        addr_space="Shared",
                    )
                    for k in range(NCH)
                ]
                for l in range(2)
            ]
            h_ch_handles = [[h_ch[l][k].tensor for k in range(NCH)] for l in range(2)]
            z_ch = [
                dpool.tile([shq[k], D], dt.bfloat16, name=f"z_ch{k}")
                for k in range(NCH)
            ]
            st_in = [
                dpool.tile([D, 2], dt.float32, name=f"st_in{l}") for l in range(3)
            ]
            st_out = [
                dpool.tile([D, 2], dt.float32, name=f"st_out{l}")
                for l in range(3)
            ]

            # ---- constants to SBUF ----
            def load(shape, src_ap, dtp=dt.float32, name=None):
                t = cpool.tile(list(shape), dtp, name=name)
                nc.sync.dma_start(out=t[:], in_=src_ap)
                return t

            KTC = (KT + CALL_KT - 1) // CALL_KT
            idx_tiles = []
            for k in range(KTC):
                w = min(CALL_KT, KT - k * CALL_KT)
                t = cpool.tile([128, CALL_KT], dt.int32, name=f"ix{k}")
                nc.sync.dma_start(
                    out=t[:, :w], in_=idx_d[:, k * CALL_KT : k * CALL_KT + w]
                )
                idx_tiles.append(t)
            rel_sb = load((128, KT), rel_d[:], name="rel_sb")
            deg2_sb = load(
                (128, ncolg * GW), deg2_d[:], dt.bfloat16, name="deg2_sb"
            )
            iota_sb = load((128, 128), iota_d[:], dt.bfloat16, name="iota_sb")
            w1_sb = [load((D, D), w1_d[l][:], name=f"w1sb{l}") for l in range(3)]
            w2_sb = [load((D, D), w2_d[l][:], name=f"w2sb{l}") for l in range(3)]
            b1_sb = [load((D, 1), b1_d[l][:], name=f"b1sb{l}") for l in range(3)]
            b2_sb = [load((D, 1), b2_d[l][:], name=f"b2sb{l}") for l in range(3)]
            gb_sb = load((D, 6), gb_d[:], name="gb_sb")
            w1r0 = cpool.tile([D, D], dt.float32, name="w1r0")
            nc.any.tensor_copy(out=w1r0[:], in_=w1_sb[0][:])
            w2r = []
            for l in range(3):
                t = cpool.tile([D, D], dt.float32, name=f"w2r{l}")
                nc.any.tensor_copy(out=t[:], in_=w2_sb[l][:])
                w2r.append(t)
            ident = cpool.tile([128, 128], dt.bfloat16, name="ident")
            make_identity(nc, ident[:])
            ident32 = cpool.tile([128, 128], dt.float32, name="ident32")
            make_identity(nc, ident32[:])

            # persistent small tiles
            s_all = cpool.tile([D, 3], dt.float32, name="s_all")
            t_all = cpool.tile([D, 3], dt.float32, name="t_all")
            w1s_sb = [
                cpool.tile([D, D], dt.float32, name=f"w1s{l}") for l in (1, 2)
            ]
            u_sb = [cpool.tile([1, D], dt.float32, name=f"u{l}") for l in (1, 2)]
            ub_sb = [
                cpool.tile([D, D], dt.bfloat16, name=f"ub{l}") for l in (1, 2)
            ]
            ones_row = cpool.tile([1, D], dt.float32, name="ones_row")
            nc.gpsimd.memset(ones_row[:], 1.0)
            ssum = cpool.tile([128, NG], dt.float32, name="ssum")
            ssq = cpool.tile([128, NG], dt.float32, name="ssq")
            sq_scr = cpool.tile([128, GW], dt.float32, name="sq_scr")
            stat_scr = cpool.tile([128, 8], dt.float32, name="stat_scr")
            pt_all = [
                cpool.tile([128, GPC], dt.float32, name=f"pt{l}")
                for l in range(3)
            ]

            def compute_fold(l):
                """Load layer-l AR'd stats; fill s_all/t_all col l and (for
                l<2) w1s_sb/u_sb of layer l+1."""
                st = statpool.tile([D, 2], dt.float32, name="st_ld")
                nc.sync.dma_start(out=st[:], in_=st_out[l][:])
                mu = stat_scr[:, 0:1]
                msq = stat_scr[:, 1:2]
                var = stat_scr[:, 2:3]
                rstd = stat_scr[:, 3:4]
                smu = stat_scr[:, 4:5]
                nc.vector.tensor_scalar_mul(mu, st[:, 0:1], inv_n)
                nc.vector.tensor_scalar_mul(msq, st[:, 1:2], inv_n)
                nc.vector.tensor_tensor(out=var, in0=mu, in1=mu, op=Alu.mult)
                nc.vector.tensor_tensor(
                    out=var, in0=msq, in1=var, op=Alu.subtract
                )
                veps = stat_scr[:, 6:7]
                nc.vector.tensor_scalar_add(veps, var, EPS)
                std = stat_scr[:, 5:6]
                nc.scalar.activation(std, veps, Act.Sqrt)
                nc.vector.reciprocal(rstd, std)
                scol = s_all[:, l : l + 1]
                tcol = t_all[:, l : l + 1]
                nc.vector.tensor_tensor(
                    out=scol, in0=gb_sb[:, 2 * l : 2 * l + 1], in1=rstd,
                    op=Alu.mult,
                )
                nc.vector.tensor_tensor(out=smu, in0=scol, in1=mu, op=Alu.mult)
                nc.vector.tensor_tensor(
                    out=tcol, in0=gb_sb[:, 2 * l + 1 : 2 * l + 2], in1=smu,
                    op=Alu.subtract,
                )
                if l < 2:
                    ln = l + 1
                    nc.vector.tensor_scalar(
                        out=w1s_sb[ln - 1][:], in0=w1_sb[ln][:], scalar1=scol,
                        scalar2=None, op0=Alu.mult,
                    )
                    ups = trpool.tile([1, D], dt.float32, name="ups", tag="tr")
                    nc.tensor.matmul(
                        ups[:], lhsT=tcol, rhs=w1_sb[ln][:], start=True,
                        stop=True,
                    )
                    nc.any.tensor_copy(out=u_sb[ln - 1][:], in_=ups[:])
                    ubp = trpool.tile([D, D], dt.float32, name="ubp", tag="tr")
                    nc.tensor.matmul(
                        ubp[:], lhsT=ones_row[:], rhs=u_sb[ln - 1][:],
                        start=True, stop=True,
                    )
                    nc.any.tensor_copy(out=ub_sb[ln - 1][:], in_=ubp[:])

            ag_insts = [[], []]
            for layer in range(3):
                tbl_ap = x_tbl_d if layer == 0 else h_ch[layer - 1][0][:]
                if layer > 0:
                    compute_fold(layer - 1)
                lhs1 = w1r0 if layer == 0 else w1s_sb[layer - 1]
                pt = pt_all[layer]

                ecur = [None]
                ecall = [-1]
                first_gather = [True]
                lyr = layer

                def e_slice(t):
                    call = t // CALL_KT
                    if call != ecall[0]:
                        w = min(CALL_KT, KT - call * CALL_KT)
                        et = epool.tile(
                            [128, CALL_KT * 128], dt.bfloat16, name="ebuf"
                        )
                        gi = nc.gpsimd.indirect_dma_start(
                            out=et[:, : w * 128],
                            out_offset=None,
                            in_=tbl_ap,
                            in_offset=bass.IndirectOffsetOnAxis(
                                ap=idx_tiles[call][:, :w],
                                axis=0,
                            ),
                        )
                        if first_gather[0]:
                            first_gather[0] = False
                            if lyr > 0:
                                for agi in ag_insts[lyr - 1]:
                                    add_dep_helper(
                                        getattr(gi, "ins", gi),
                                        getattr(agi, "ins", agi),
                                        reason="gather waits h-chunk AllGather",
                                    )
                        ecur[0], ecall[0] = et, call
                    p = t - call * CALL_KT
                    return ecur[0][:, p * 128 : (p + 1) * 128]

                for g in range(NG):
                    blo = g * GRP_BLKS
                    bhi = min(blo + GRP_BLKS, NB)
                    W = (bhi - blo) * 128
                    zin = zinpool.tile([128, GW], dt.float32, name="zin")
                    for b in range(blo, bhi):
                        agg = aggpool.tile([128, 128], dt.float32, name="agg")
                        nk = int(hd.blk_nk[b])
                        t0 = int(hd.blk_kt0[b])
                        for j in range(nk):
                            esl = e_slice(t0 + j)
                            s_t = spool.tile(
                                [128, 128], dt.bfloat16, name="s_t"
                            )
                            nc.vector.tensor_scalar(
                                out=s_t[:], in0=iota_sb[:],
                                scalar1=rel_sb[:, t0 + j : t0 + j + 1],
                                scalar2=None, op0=Alu.is_equal,
                            )
                            nc.tensor.matmul(
                                agg[:], lhsT=esl, rhs=s_t[:],
                                start=(j == 0), stop=(j == nk - 1),
                            )
                        co = (b - blo) * 128
                        nc.any.tensor_copy(
                            out=zin[:, co : co + 128], in_=agg[:]
                        )
                    # ---- MLP on the group (transposed space, fp32r) ----
                    m1 = m1pool.tile([128, GW], dt.float32, name="m1")
                    nc.tensor.matmul(
                        m1[:, :W], lhsT=lhs1[:], rhs=zin[:, :W],
                        start=True, stop=(layer == 0),
                    )
                    if layer > 0:
                        dp = (g % 2) * 64
                        dc = (g // 2) * GW
                        nc.tensor.matmul(
                            m1[:, :W], lhsT=ub_sb[layer - 1][dp : dp + 1, :],
                            rhs=deg2_sb[dp : dp + 1, dc : dc + W],
                            start=False, stop=True,
                        )
                    z1 = zmidpool.tile([128, GW], dt.float32, name="z1")
                    nc.scalar.activation(
                        z1[:, :W], m1[:, :W], Act.Relu, bias=b1_sb[layer][:]
                    )
                    m2 = m2pool.tile([128, GW], dt.float32, name="m2")
                    nc.tensor.matmul(
                        m2[:, :W], lhsT=w2r[layer][:], rhs=z1[:, :W],
                        start=True, stop=True,
                    )
                    z2 = zmidpool.tile([128, GW], dt.bfloat16, name="z2")
                    c0 = g * GW
                    wr = min(W, max(0, SLOTS - c0))  # stat cols (real+dup)
                    if wr > 0:
                        nc.scalar.activation(
                            z2[:, :wr], m2[:, :wr], Act.Relu,
                            bias=b2_sb[layer][:], accum_out=ssum[:, g : g + 1],
                        )
                    if wr < W:
                        nc.scalar.activation(
                            z2[:, wr:W], m2[:, wr:W], Act.Relu,
                            bias=b2_sb[layer][:],
                        )
                    if wr > 0:
                        nc.scalar.activation(
                            sq_scr[:, :wr], z2[:, :wr], Act.Square,
                            accum_out=ssq[:, g : g + 1],
                        )
                    # ---- on-the-fly pooling (raw m2, fp32; relu+b2 at end) ----
                    pc1 = min(c0 + W, SLOTS)
                    if c0 < pc1:
                        gfirst = (c0 + GS - 1) // GS
                        a = gfirst * GS - c0
                        gend = pc1 // GS
                        nfull = gend - gfirst
                        if nfull > 0:
                            nc.vector.tensor_reduce(
                                out=pt[:, gfirst:gend],
                                in_=m2[:, a : a + nfull * GS].rearrange(
                                    "p (g s) -> p g s", s=GS
                                ),
                                axis=mybir.AxisListType.X, op=Alu.max,
                            )
                        if a > 0:  # left partial graph gfirst-1
                            la = min(a, pc1 - c0)
                            tmpm = stat_scr[:, 7:8]
                            nc.vector.tensor_reduce(
                                out=tmpm, in_=m2[:, 0:la],
                                axis=mybir.AxisListType.X, op=Alu.max,
                            )
                            gl = gfirst - 1
                            nc.vector.tensor_tensor(
                                out=pt[:, gl : gl + 1],
                                in0=pt[:, gl : gl + 1], in1=tmpm, op=Alu.max,
                            )
                        r0 = a + max(0, gend - gfirst) * GS
                        if gend >= gfirst and c0 + r0 < pc1:
                            # right partial graph gend (first touch)
                            nc.vector.tensor_reduce(
                                out=pt[:, gend : gend + 1],
                                in_=m2[:, r0 : pc1 - c0],
                                axis=mybir.AxisListType.X, op=Alu.max,
                            )
                    # ---- transpose to node-major for the h table ----
                    if layer < 2:
                        for i in range(W // 128):
                            trp = trpool.tile(
                                [128, 128], dt.bfloat16, name="trp", tag="tr"
                            )
                            nc.tensor.transpose(
                                trp[:], z2[:, i * 128 : (i + 1) * 128],
                                ident[:],
                            )
                            rm = rmpool.tile([128, 128], dt.bfloat16, name="rm")
                            nc.any.tensor_copy(out=rm[:], in_=trp[:])
                            b2i = blo + i
                            kch = 0
                            while cb[kch + 1] <= b2i:
                                kch += 1
                            lr0 = (b2i - cb[kch]) * 128
                            nc.sync.dma_start(
                                out=z_ch[kch][lr0 : lr0 + 128, :], in_=rm[:]
                            )
                        # launch chunk AllGather as soon as its blocks done
                        for kch in range(NCH):
                            if (cb[kch + 1] - 1) // GRP_BLKS == g:
                                agi = nc.gpsimd.collective_compute(
                                    "AllGather", mybir.AluOpType.bypass,
                                    replica_groups=[list(range(C))],
                                    ins=[z_ch[kch].opt()],
                                    outs=[h_ch[layer][kch].opt()],
                                )
                                ag_insts[layer].append(agi)

                # ---- stats reduce + AllReduce ----
                sp = statpool.tile([D, 2], dt.float32, name="sp")
                nc.vector.tensor_reduce(
                    out=sp[:, 0:1], in_=ssum[:, :NG],
                    axis=mybir.AxisListType.X, op=Alu.add,
                )
                nc.vector.tensor_reduce(
                    out=sp[:, 1:2], in_=ssq[:, :NG],
                    axis=mybir.AxisListType.X, op=Alu.add,
                )
                nc.sync.dma_start(out=st_in[layer][:], in_=sp[:])
                nc.gpsimd.collective_compute(
                    "AllReduce", mybir.AluOpType.add,
                    replica_groups=[list(range(C))],
                    ins=[st_in[layer].opt()], outs=[st_out[layer].opt()],
                )
                pass

            # ---- output: affine + transpose + store ----
            compute_fold(2)
            out_big = cpool.tile(
                [128, n_pool_chunks * 3 * D], dt.float32, name="out_big"
            )
            with tc.tile_pool(name="poolt", bufs=2) as ptpool:
                for l in range(3):
                    # pooled z2 = relu(max(m2) + b2); then BN affine
                    pre = ptpool.tile([128, GPC], dt.float32, name="pre")
                    nc.scalar.activation(
                        pre[:], pt_all[l][:], Act.Relu, bias=b2_sb[l][:]
                    )
                    pta = ptpool.tile([128, GPC], dt.float32, name="pta")
                    nc.vector.tensor_scalar(
                        out=pta[:], in0=pre[:],
                        scalar1=s_all[:, l : l + 1],
                        scalar2=t_all[:, l : l + 1], op0=Alu.mult, op1=Alu.add,
                    )
                    for ch in range(n_pool_chunks):
                        rows = (
                            128 if ch < n_pool_chunks - 1 else last_chunk_rows
                        )
                        trp = trpool.tile(
                            [128, 128], dt.float32, name="trpo", tag="tr"
                        )
                        nc.tensor.transpose(
                            trp[:rows, :],
                            pta[:, ch * 128 : ch * 128 + rows], ident32[:],
                        )
                        nc.any.tensor_copy(
                            out=out_big[
                                :rows, ch * 3 * D + l * D : ch * 3 * D
                                + (l + 1) * D
                            ],
                            in_=trp[:rows, :],
                        )
            for ch in range(n_pool_chunks):
                rows = 128 if ch < n_pool_chunks - 1 else last_chunk_rows
                nc.sync.dma_start(
                    out=out_d[ch * 128 : ch * 128 + rows, :],
                    in_=out_big[:rows, ch * 3 * D : (ch + 1) * 3 * D],
                )

    nc.compile()
    for l in range(2):
        base = None
        for k in range(N_CHUNKS):
            mls = nc.lookup_mls(h_ch_handles[l][k])
            addr = mls.memorylocations[0].addr
            shq_k = mls.tensor_shape[0]
            if base is not None:
                assert addr == base, (
                    f"h chunks not contiguous at l={l} k={k}: {addr} != {base}"
                )
            base = addr + shq_k * DIM * 2
    return nc, input_names


def make_in_maps(hd: HostData, inputs: dict, input_names):
    iota = np.tile(np.arange(128, dtype=np.float32), (128, 1)).astype(np_bf16)
    gb = np.zeros((DIM, 6), dtype=np.float32)
    for l in range(3):
        gb[:, 2 * l] = inputs["gamma"][l]
        gb[:, 2 * l + 1] = inputs["beta"][l]
    shared = {
        "x_tbl": hd.x_tbl,
        "iota": np.ascontiguousarray(iota),
        "gb": gb,
    }
    for l in range(3):
        w = np.zeros((DIM, DIM), dtype=np.float32)
        wl = inputs[f"w1_{l}"]
        w[: wl.shape[0], :] = wl
        shared[f"w1_{l}"] = w
        shared[f"w2_{l}"] = np.ascontiguousarray(
            inputs[f"w2_{l}"].astype(np.float32)
        )
        shared[f"b1_{l}"] = inputs[f"b1_{l}"].astype(np.float32).reshape(-1, 1)
        shared[f"b2_{l}"] = inputs[f"b2_{l}"].astype(np.float32).reshape(-1, 1)
    in_maps = []
    for c in range(N_CORES):
        m = dict(shared)
        m["idx"] = hd.idx_sb[c]
        m["rel"] = hd.rel_sb[c]
        m["deg2"] = hd.deg2[c]
        assert set(m.keys()) == set(input_names)
        in_maps.append(m)
    return in_maps


def _run_sharded_timed(nc, in_maps, n_cores, iters=10, warmup=2):
    """Execute the compiled Bass module via PJRT with device-resident inputs,
    timing `iters` back-to-back dispatches (excludes input upload/compile)."""
    import time

    import jax
    from jax.sharding import Mesh, NamedSharding, PartitionSpec
    from jax.experimental.shard_map import shard_map

    import concourse.mybir as mybir
    from concourse import bass2jax

    bass2jax.install_neuronx_cc_hook()
    partition_name = (
        nc.partition_id_tensor.name if nc.partition_id_tensor else None
    )
    in_names, out_names, out_avals, zero_outs = [], [], [], []
    for alloc in nc.m.functions[0].allocations:
        if not isinstance(alloc, mybir.MemoryLocationSet):
            continue
        name = alloc.memorylocations[0].name
        if alloc.kind == "ExternalInput":
            if name != partition_name:
                in_names.append(name)
        elif alloc.kind == "ExternalOutput":
            out_names.append(name)
            shape = tuple(alloc.tensor_shape)
            dtp = mybir.dt.np(alloc.dtype)
            out_avals.append(jax.core.ShapedArray(shape, dtp))
            zero_outs.append(np.zeros(shape, dtp))
    n_params, n_outs = len(in_names), len(out_avals)
    in_names.extend(out_names)
    if partition_name is not None:
        in_names.append(partition_name)
    donate = tuple(range(n_params, n_params + n_outs))

    def _body(*args):
        operands = list(args)
        if partition_name is not None:
            operands.append(bass2jax.partition_id_tensor())
        outs = bass2jax._bass_exec_p.bind(
            *operands,
            out_avals=tuple(out_avals),
            in_names=tuple(in_names),
            out_names=tuple(out_names),
            lowering_input_output_aliases=(),
            sim_require_finite=True,
            sim_require_nnan=True,
            nc=nc,
        )
        return tuple(outs)

    devices = jax.devices()[:n_cores]
    mesh = Mesh(np.asarray(devices), ("core",))
    pspec = PartitionSpec("core")
    in_specs = (pspec,) * (n_params + n_outs)
    sharded = jax.jit(
        shard_map(
            _body, mesh=mesh, in_specs=in_specs,
            out_specs=(pspec,) * len(out_names), check_rep=False,
        ),
        donate_argnums=donate, keep_unused=True,
    )
    shd = NamedSharding(mesh, pspec)
    per_core = [
        [np.asarray(m[name]) for name in in_names[:n_params]] for m in in_maps
    ]
    dev_in = [
        jax.device_put(
            np.concatenate([per_core[c][i] for c in range(n_cores)], axis=0),
            shd,
        )
        for i in range(n_params)
    ]
    n_calls = warmup + (iters if iters else 0)
    zsets = [
        [
            jax.device_put(
                np.zeros((n_cores * z.shape[0], *z.shape[1:]), z.dtype), shd
            )
            for z in zero_outs
        ]
        for _ in range(max(n_calls, 1))
    ]
    outs = None
    for i in range(warmup):
        outs = sharded(*dev_in, *zsets[i])
        jax.block_until_ready(outs)
    dt = None
    if iters:
        t0 = time.perf_counter()
        ress = [sharded(*dev_in, *zsets[warmup + i]) for i in range(iters)]
        jax.block_until_ready(ress)
        dt = (time.perf_counter() - t0) / iters
        outs = ress[-1]
    if outs is None:
        outs = sharded(*dev_in, *zsets[0])
    results = [
        {
            name: np.asarray(outs[i]).reshape(n_cores, *out_avals[i].shape)[c]
            for i, name in enumerate(out_names)
        }
        for c in range(n_cores)
    ]
    return results, dt


def run(inputs: dict, timed: bool = False):
    x = np.asarray(inputs["x"])
    ei = np.asarray(inputs["edge_index"])
    batch = np.asarray(inputs["batch"])
    hd = prep_host(x, ei, batch)
    nc, input_names = build_program(hd)
    in_maps = make_in_maps(hd, inputs, input_names)
    results, dt = _run_sharded_timed(
        nc, in_maps, N_CORES,
        iters=(10 if timed else 0), warmup=(2 if timed else 1),
    )
    outs = [results[c]["pooled"] for c in range(N_CORES)]
    full = np.concatenate(outs, axis=0).astype(np.float32)
    return full, dt


def kernel(**inputs) -> np.ndarray:
    out, _ = run(inputs, timed=False)
    return out


# revision 30
# speedup vs baseline: 1.1830x; 1.0534x over previous
"""GIN (3-layer) message-passing kernel for Trainium2, 8 NeuronCores.

Strategy (graph-partition data parallel):
  - Graphs are assigned to cores by id: core c owns graphs [c*750, (c+1)*750).
    Nodes are renumbered so each graph occupies a fixed GS-slot stride
    (GS = max graph size; the reference batch yields 49-51-node graphs);
    slots beyond a graph's size duplicate the graph's first node (same
    in-edges, same degree), so the padded slot computes exactly the same z
    as the duplicated node and segment-max pooling is a uniform-width
    reduce, core-local.  BN stats are means over the padded population
    (a <1% reweighting, folded into the divisor).
  - Edges sharded by destination core, sorted by local dst slot, grouped
    into 128-slot blocks, padded per block to 128-edge "k-tiles" (structure
    shared across cores so the SPMD program is identical).  The GIN self
    term is added from an SBUF-resident copy of the previous layer's z2
    (feat-major), so self-loops are never gathered.
  - Aggregation: per k-tile indirect-DMA gathers fetch h[src] rows (bf16,
    256B) from the shared-HBM h table; a one-hot matrix S (vector iota/
    is_equal, bf16) right-multiplies the gathered tile on the tensor
    engine, accumulating aggT[feat, slot] in PSUM per block.  (The SWDGE
    offset walker corrupts multi-column offset APs, so gathers are one
    k-tile per call.)
  - MLP runs in transposed space (feat on partitions): aggregation matmuls
    in bf16, the two MLP matmuls in fp32r; BatchNorm of the previous layer
    is folded into the next layer's first matmul (w1 row-scaled by s plus
    a rank-1 (w1^T t) x deg correction), so h tables stay un-normalized.
    BN statistics come free from activation accum_out (fp32); a 1KB
    AllReduce shares them.
  - The h table lives in the shared DRAM scratchpad (all 8 cores share
    HBM) split into 4 row-chunks per layer; each chunk's AllGather is
    issued as soon as its blocks are computed, overlapping the collective
    with the remaining compute.  Explicit deps order the next layer's
    gathers after all chunks.
  - Pooling on the fly: per-group segment-max over the fp32 m2 PSUM
    (relu/bias applied once at the end; max commutes with the monotone
    affine), then the BN affine, transpose, concat per-core output.
Host assembles the 8 per-core [750, 384] outputs into the full [6000, 384].
"""

import sys

sys.path.insert(0, "/opt/trn_rl_repo")

import math
from dataclasses import dataclass

import numpy as np

try:
    from ml_dtypes import bfloat16 as np_bf16
except ImportError:  # pragma: no cover
    import jax.numpy as _jnp

    np_bf16 = _jnp.bfloat16

N_GRAPHS = 6000
N_CORES = 8
IN_DIM = 77
DIM = 128
EPS = 1e-5
CALL_KT = 1  # k-tiles per indirect gather call
GRP_BLKS = 4  # 128-slot blocks per MLP group (=512 cols)
N_CHUNKS = 4  # h-table chunks for overlapped AllGathers


@dataclass
class HostData:
    gs: int  # padded graph stride (max graph size)
    gpc: int  # graphs per core
    slots: int  # real+dup slots per core (gpc*gs)
    nb: int  # 128-slot blocks per core
    shp: int  # padded slots per core (nb*128)
    kt_total: int
    blk_kt0: np.ndarray  # [nb] first k-tile of each block
    blk_nk: np.ndarray  # [nb] k-tiles per block
    idx_sb: list  # per core [128, KT] int32 gather row ids
    rel_sb: list  # per core [128, KT] f32 dst-in-block (or -1 pad)
    deg2: list  # per core [128, ncolg*GW] bf16 (rank-1 fold layout)
    x_tbl: np.ndarray  # [tbl, 128] bf16
    cb: np.ndarray  # [N_CHUNKS+1] chunk boundaries in blocks

    @property
    def tbl(self):
        return N_CORES * self.shp

    @property
    def ng(self):
        return (self.nb + GRP_BLKS - 1) // GRP_BLKS


def prep_host(x: np.ndarray, edge_index: np.ndarray, batch: np.ndarray) -> HostData:
    C = N_CORES
    N = x.shape[0]
    batch = batch.astype(np.int64)
    sizes = np.bincount(batch, minlength=N_GRAPHS)
    assert sizes.min() >= 1
    starts = np.concatenate([[0], np.cumsum(sizes)[:-1]])
    GS = int(sizes.max())
    GPC = N_GRAPHS // C
    SLOTS = GPC * GS
    NB = (SLOTS + 127) // 128
    SHP = NB * 128
    TBL = C * SHP

    # chunked table layout: chunk k holds blocks [cb[k], cb[k+1]) of all cores
    nb_ch = [NB // N_CHUNKS + (1 if i < NB % N_CHUNKS else 0) for i in range(N_CHUNKS)]
    cb = np.concatenate([[0], np.cumsum(nb_ch)]).astype(np.int64)
    shq = [int(n) * 128 for n in nb_ch]
    chunk_base = np.concatenate([[0], np.cumsum([C * s for s in shq])]).astype(np.int64)

    def slot_to_row(core, slot):
        blk = slot >> 7
        k = np.searchsorted(cb[1:], blk, side="right")
        return chunk_base[k] + core * np.asarray(shq)[k] + (slot - cb[k] * 128)

    # node -> (core, local slot, global table row)
    g_of = batch
    pos = np.arange(N, dtype=np.int64) - starts[g_of]
    core_of = g_of // GPC
    slot_loc = (g_of - core_of * GPC) * GS + pos
    row_of = slot_to_row(core_of, slot_loc).astype(np.int64)

    src = edge_index[0].astype(np.int64)
    dst = edge_index[1].astype(np.int64)

    # destination-side entries: (core, dslot, src_row)
    e_core = [core_of[dst], core_of]
    e_dslot = [slot_loc[dst], slot_loc]
    e_srow = [row_of[src], row_of]  # real edges + self edges

    # duplicate slots: graph g's pad slots [size_g, GS) copy n0 = starts[g]
    n0_edges = np.where(dst == starts[g_of[dst]])[0]  # edges into any n0
    n0_g = g_of[dst[n0_edges]]
    max_pad = GS - int(sizes.min())
    for j in range(max_pad):
        gsel_mask = sizes + j < GS  # graphs needing pad slot at size_g + j
        # in-edges of n0 for selected graphs
        em = gsel_mask[n0_g]
        gg = n0_g[em]
        pc = gg // GPC
        ps = (gg - pc * GPC) * GS + sizes[gg] + j
        e_core.append(pc)
        e_dslot.append(ps)
        e_srow.append(row_of[src[n0_edges[em]]])
        # the duplicated node's self term: edge from n0's row
        gsel = np.where(gsel_mask)[0]
        pc2 = gsel // GPC
        e_core.append(pc2)
        e_dslot.append((gsel - pc2 * GPC) * GS + sizes[gsel] + j)
        e_srow.append(row_of[starts[gsel]])

    e_core = np.concatenate(e_core)
    e_dslot = np.concatenate(e_dslot)
    e_srow = np.concatenate(e_srow)

    per_core = []
    cnts = np.zeros((C, NB), dtype=np.int64)
    for c in range(C):
        m = e_core == c
        dl_c, sr_c = e_dslot[m], e_srow[m]
        order = np.argsort(dl_c, kind="stable")
        dl_c, sr_c = dl_c[order], sr_c[order]
        blk = dl_c >> 7
        cnts[c] = np.bincount(blk, minlength=NB)
        per_core.append((sr_c, dl_c, blk))

    blk_nk = (cnts.max(axis=0) + 127) // 128  # shared k-tile structure
    blk_nk = np.maximum(blk_nk, 1)
    blk_kt0 = np.concatenate([[0], np.cumsum(blk_nk)[:-1]])
    KT = int(blk_nk.sum())
    k_pad = KT * 128

    NG = (NB + GRP_BLKS - 1) // GRP_BLKS
    GW = GRP_BLKS * 128
    ncolg = (NG + 1) // 2

    idx_sb, rel_sb, deg2 = [], [], []
    for c in range(C):
        sr_c, dl_c, blk = per_core[c]
        bstart = np.concatenate([[0], np.cumsum(cnts[c])[:-1]])
        p = np.arange(len(sr_c)) - bstart[blk]
        slot = blk_kt0[blk] * 128 + p
        idx_arr = np.zeros(k_pad, dtype=np.int32)
        rel_arr = np.full(k_pad, -1.0, dtype=np.float32)
        idx_arr[slot] = sr_c.astype(np.int32)
        rel_arr[slot] = (dl_c & 127).astype(np.float32)
        idx_sb.append(np.ascontiguousarray(idx_arr.reshape(KT, 128).T))
        rel_sb.append(np.ascontiguousarray(rel_arr.reshape(KT, 128).T))

        # per-slot degree (= in-edges incl self) for the rank-1 BN-fold matmul
        deg_p = np.bincount(dl_c, minlength=SHP).astype(np.float32)
        d2 = np.zeros((128, ncolg * GW), dtype=np.float32)
        for g in range(NG):
            seg = deg_p[g * GW : (g + 1) * GW]
            d2[(g % 2) * 64, (g // 2) * GW : (g // 2) * GW + len(seg)] = seg
        deg2.append(d2.astype(np_bf16))

    x_tbl = np.zeros((TBL, 128), dtype=np_bf16)
    x_tbl[row_of, :IN_DIM] = x.astype(np_bf16)

    return HostData(GS, GPC, SLOTS, NB, SHP, KT, blk_kt0, blk_nk, idx_sb, rel_sb, deg2, x_tbl, cb)


def build_program(hd: HostData):
    """Returns (nc, input_names)."""
    import concourse.bass as bass
    import concourse.mybir as mybir
    import concourse.tile as tile
    from concourse import bacc
    from concourse.masks import make_identity
    from concourse.tile_rust import add_dep_helper

    dt = mybir.dt
    Alu = mybir.AluOpType
    Act = mybir.ActivationFunctionType

    C, D = N_CORES, DIM
    NB, SHP, TBL, NG, KT = hd.nb, hd.shp, hd.tbl, hd.ng, hd.kt_total
    GW = GRP_BLKS * 128
    GS, GPC, SLOTS = hd.gs, hd.gpc, hd.slots
    ncolg = (NG + 1) // 2
    inv_n = 1.0 / (C * SLOTS)

    nc = bacc.Bacc(
        "TRN2", target_bir_lowering=False, debug=False, num_devices=C
    )

    def din(name, shape, dtp=dt.float32):
        return nc.dram_tensor(name, list(shape), dtp, kind="ExternalInput").ap()

    x_tbl_d = din("x_tbl", (TBL, D), dt.bfloat16)
    idx_d = din("idx", (128, KT), dt.int32)
    rel_d = din("rel", (128, KT))
    deg2_d = din("deg2", (128, ncolg * GW), dt.bfloat16)
    iota_d = din("iota", (128, 128), dt.bfloat16)
    w1_d = [din(f"w1_{l}", (D, D)) for l in range(3)]
    w2_d = [din(f"w2_{l}", (D, D)) for l in range(3)]
    b1_d = [din(f"b1_{l}", (D, 1)) for l in range(3)]
    b2_d = [din(f"b2_{l}", (D, 1)) for l in range(3)]
    gb_d = din("gb", (D, 6))  # cols: g0 b0 g1 b1 g2 b2
    out_d = nc.dram_tensor(
        "pooled", [GPC, 3 * D], dt.float32, kind="ExternalOutput"
    ).ap()

    input_names = (
        ["x_tbl", "idx", "rel", "deg2", "iota"]
        + [f"w1_{l}" for l in range(3)]
        + [f"w2_{l}" for l in range(3)]
        + [f"b1_{l}" for l in range(3)]
        + [f"b2_{l}" for l in range(3)]
        + ["gb"]
    )

    n_pool_chunks = (GPC + 127) // 128
    last_chunk_rows = GPC - (n_pool_chunks - 1) * 128

    with tile.TileContext(nc) as tc:
        with (
            tc.tile_pool(name="const", bufs=1) as cpool,
            tc.tile_pool(name="ebuf", bufs=10) as epool,
            tc.tile_pool(name="spool", bufs=8) as spool,
            tc.tile_pool(name="zin", bufs=2) as zinpool,
            tc.tile_pool(name="zmid", bufs=2) as zmidpool,
            tc.tile_pool(name="rm", bufs=3) as rmpool,
            tc.tile_pool(name="stat", bufs=1) as statpool,
            tc.tile_pool(name="agg_ps", bufs=2, space="PSUM") as aggpool,
            tc.tile_pool(name="m1_ps", bufs=2, space="PSUM") as m1pool,
            tc.tile_pool(name="m2_ps", bufs=2, space="PSUM") as m2pool,
            tc.tile_pool(name="tr_ps", bufs=2, space="PSUM") as trpool,
            tc.tile_pool(name="dram", bufs=1, space="DRAM") as dpool,
        ):
            # ---- DRAM intermediates ----
            cb = [int(v) for v in hd.cb]
            NCH = N_CHUNKS
            shq = [(cb[k + 1] - cb[k]) * 128 for k in range(NCH)]
            # h chunks per layer; consecutive allocation => contiguous region
            h_ch = [
                [
                    dpool.tile(
                        [C * shq[k], D], dt.bfloat16, name=f"h_{l}_{k}",
                        addr_space="Shared",
                    )
                    for k in range(NCH)
                ]
                for l in range(2)
            ]
            h_ch_handles = [[h_ch[l][k].tensor for k in range(NCH)] for l in range(2)]
            z_ch = [
                dpool.tile([shq[k], D], dt.bfloat16, name=f"z_ch{k}")
                for k in range(NCH)
            ]
            st_in = [
                dpool.tile([D, 2], dt.float32, name=f"st_in{l}") for l in range(3)
            ]
            st_out = [
                dpool.tile([D, 2], dt.float32, name=f"st_out{l}")
                for l in range(3)
            ]

            # ---- constants to SBUF ----
            def load(shape, src_ap, dtp=dt.float32, name=None):
                t = cpool.tile(list(shape), dtp, name=name)
                nc.sync.dma_start(out=t[:], in_=src_ap)
                return t

            idx_sb = load((128, KT), idx_d[:], dt.int32, name="idx_sb")
            rel_sb = load((128, KT), rel_d[:], name="rel_sb")
            deg2_sb = load(
                (128, ncolg * GW), deg2_d[:], dt.bfloat16, name="deg2_sb"
            )
            iota_sb = load((128, 128), iota_d[:], dt.bfloat16, name="iota_sb")
            w1_sb = [load((D, D), w1_d[l][:], name=f"w1sb{l}") for l in range(3)]
            w2_sb = [load((D, D), w2_d[l][:], name=f"w2sb{l}") for l in range(3)]
            b1_sb = [load((D, 1), b1_d[l][:], name=f"b1sb{l}") for l in range(3)]
            b2_sb = [load((D, 1), b2_d[l][:], name=f"b2sb{l}") for l in range(3)]
            gb_sb = load((D, 6), gb_d[:], name="gb_sb")
            w1r0 = cpool.tile([D, D], dt.float32, name="w1r0")
            nc.any.tensor_copy(out=w1r0[:], in_=w1_sb[0][:])
            w2r = []
            for l in range(3):
                t = cpool.tile([D, D], dt.float32, name=f"w2r{l}")
                nc.any.tensor_copy(out=t[:], in_=w2_sb[l][:])
                w2r.append(t)
            ident = cpool.tile([128, 128], dt.bfloat16, name="ident")
            make_identity(nc, ident[:])
            ident32 = cpool.tile([128, 128], dt.float32, name="ident32")
            make_identity(nc, ident32[:])

            # persistent small tiles
            s_all = cpool.tile([D, 3], dt.float32, name="s_all")
            t_all = cpool.tile([D, 3], dt.float32, name="t_all")
            w1s_sb = [
                cpool.tile([D, D], dt.float32, name=f"w1s{l}") for l in (1, 2)
            ]
            u_sb = [cpool.tile([1, D], dt.float32, name=f"u{l}") for l in (1, 2)]
            ub_sb = [
                cpool.tile([D, D], dt.bfloat16, name=f"ub{l}") for l in (1, 2)
            ]
            ones_row = cpool.tile([1, D], dt.float32, name="ones_row")
            nc.gpsimd.memset(ones_row[:], 1.0)
            ssum = cpool.tile([128, NG], dt.float32, name="ssum")
            ssq = cpool.tile([128, NG], dt.float32, name="ssq")
            sq_scr = cpool.tile([128, GW], dt.float32, name="sq_scr")
            stat_scr = cpool.tile([128, 8], dt.float32, name="stat_scr")
            pt_all = [
                cpool.tile([128, GPC], dt.float32, name=f"pt{l}")
                for l in range(3)
            ]

            def compute_fold(l):
                """Load layer-l AR'd stats; fill s_all/t_all col l and (for
                l<2) w1s_sb/u_sb of layer l+1."""
                st = statpool.tile([D, 2], dt.float32, name="st_ld")
                nc.sync.dma_start(out=st[:], in_=st_out[l][:])
                mu = stat_scr[:, 0:1]
                msq = stat_scr[:, 1:2]
                var = stat_scr[:, 2:3]
                rstd = stat_scr[:, 3:4]
                smu = stat_scr[:, 4:5]
                nc.vector.tensor_scalar_mul(mu, st[:, 0:1], inv_n)
                nc.vector.tensor_scalar_mul(msq, st[:, 1:2], inv_n)
                nc.vector.tensor_tensor(out=var, in0=mu, in1=mu, op=Alu.mult)
                nc.vector.tensor_tensor(
                    out=var, in0=msq, in1=var, op=Alu.subtract
                )
                veps = stat_scr[:, 6:7]
                nc.vector.tensor_scalar_add(veps, var, EPS)
                std = stat_scr[:, 5:6]
                nc.scalar.activation(std, veps, Act.Sqrt)
                nc.vector.reciprocal(rstd, std)
                scol = s_all[:, l : l + 1]
                tcol = t_all[:, l : l + 1]
                nc.vector.tensor_tensor(
                    out=scol, in0=gb_sb[:, 2 * l : 2 * l + 1], in1=rstd,
                    op=Alu.mult,
                )
                nc.vector.tensor_tensor(out=smu, in0=scol, in1=mu, op=Alu.mult)
                nc.vector.tensor_tensor(
                    out=tcol, in0=gb_sb[:, 2 * l + 1 : 2 * l + 2], in1=smu,
                    op=Alu.subtract,
                )
                if l < 2:
                    ln = l + 1
                    nc.vector.tensor_scalar(
                        out=w1s_sb[ln - 1][:], in0=w1_sb[ln][:], scalar1=scol,
                        scalar2=None, op0=Alu.mult,
                    )
                    ups = trpool.tile([1, D], dt.float32, name="ups", tag="tr")
                    nc.tensor.matmul(
                        ups[:], lhsT=tcol, rhs=w1_sb[ln][:], start=True,
                        stop=True,
                    )
                    nc.any.tensor_copy(out=u_sb[ln - 1][:], in_=ups[:])
                    ubp = trpool.tile([D, D], dt.float32, name="ubp", tag="tr")
                    nc.tensor.matmul(
                        ubp[:], lhsT=ones_row[:], rhs=u_sb[ln - 1][:],
                        start=True, stop=True,
                    )
                    nc.any.tensor_copy(out=ub_sb[ln - 1][:], in_=ubp[:])

            ag_insts = [[], []]
            for layer in range(3):
                tbl_ap = x_tbl_d if layer == 0 else h_ch[layer - 1][0][:]
                if layer > 0:
                    compute_fold(layer - 1)
                lhs1 = w1r0 if layer == 0 else w1s_sb[layer - 1]
                pt = pt_all[layer]

                ecur = [None]
                ecall = [-1]
                first_gather = [True]
                lyr = layer

                def e_slice(t):
                    call = t // CALL_KT
                    if call != ecall[0]:
                        w = min(CALL_KT, KT - call * CALL_KT)
                        et = epool.tile(
                            [128, CALL_KT * 128], dt.bfloat16, name="ebuf"
                        )
                        gi = nc.gpsimd.indirect_dma_start(
                            out=et[:, : w * 128],
                            out_offset=None,
                            in_=tbl_ap,
                            in_offset=bass.IndirectOffsetOnAxis(
                                ap=idx_sb[
                                    :, call * CALL_KT : call * CALL_KT + w
                                ],
                                axis=0,
                            ),
                        )
                        if first_gather[0]:
                            first_gather[0] = False
                            if lyr > 0:
                                for agi in ag_insts[lyr - 1]:
                                    add_dep_helper(
                                        getattr(gi, "ins", gi),
                                        getattr(agi, "ins", agi),
                                        reason="gather waits h-chunk AllGather",
                                    )
                        ecur[0], ecall[0] = et, call
                    p = t - call * CALL_KT
                    return ecur[0][:, p * 128 : (p + 1) * 128]

                for g in range(NG):
                    blo = g * GRP_BLKS
                    bhi = min(blo + GRP_BLKS, NB)
                    W = (bhi - blo) * 128
                    zin = zinpool.tile([128, GW], dt.float32, name="zin")
                    for b in range(blo, bhi):
                        agg = aggpool.tile([128, 128], dt.float32, name="agg")
                        nk = int(hd.blk_nk[b])
                        t0 = int(hd.blk_kt0[b])
                        for j in range(nk):
                            esl = e_slice(t0 + j)
                            s_t = spool.tile(
                                [128, 128], dt.bfloat16, name="s_t"
                            )
                            nc.vector.tensor_scalar(
                                out=s_t[:], in0=iota_sb[:],
                                scalar1=rel_sb[:, t0 + j : t0 + j + 1],
                                scalar2=None, op0=Alu.is_equal,
                            )
                            nc.tensor.matmul(
                                agg[:], lhsT=esl, rhs=s_t[:],
                                start=(j == 0), stop=(j == nk - 1),
                            )
                        co = (b - blo) * 128
                        nc.any.tensor_copy(
                            out=zin[:, co : co + 128], in_=agg[:]
                        )
                    # ---- MLP on the group (transposed space, fp32r) ----
                    m1 = m1pool.tile([128, GW], dt.float32, name="m1")
                    nc.tensor.matmul(
                        m1[:, :W], lhsT=lhs1[:], rhs=zin[:, :W],
                        start=True, stop=(layer == 0),
                    )
                    if layer > 0:
                        dp = (g % 2) * 64
                        dc = (g // 2) * GW
                        nc.tensor.matmul(
                            m1[:, :W], lhsT=ub_sb[layer - 1][dp : dp + 1, :],
                            rhs=deg2_sb[dp : dp + 1, dc : dc + W],
                            start=False, stop=True,
                        )
                    z1 = zmidpool.tile([128, GW], dt.float32, name="z1")
                    nc.scalar.activation(
                        z1[:, :W], m1[:, :W], Act.Relu, bias=b1_sb[layer][:]
                    )
                    m2 = m2pool.tile([128, GW], dt.float32, name="m2")
                    nc.tensor.matmul(
                        m2[:, :W], lhsT=w2r[layer][:], rhs=z1[:, :W],
                        start=True, stop=True,
                    )
                    z2 = zmidpool.tile([128, GW], dt.bfloat16, name="z2")
                    c0 = g * GW
                    wr = min(W, max(0, SLOTS - c0))  # stat cols (real+dup)
                    if wr > 0:
                        nc.scalar.activation(
                            z2[:, :wr], m2[:, :wr], Act.Relu,
                            bias=b2_sb[layer][:], accum_out=ssum[:, g : g + 1],
                        )
                    if wr < W:
                        nc.scalar.activation(
                            z2[:, wr:W], m2[:, wr:W], Act.Relu,
                            bias=b2_sb[layer][:],
                        )
                    if wr > 0:
                        nc.scalar.activation(
                            sq_scr[:, :wr], z2[:, :wr], Act.Square,
                            accum_out=ssq[:, g : g + 1],
                        )
                    # ---- on-the-fly pooling (raw m2, fp32; relu+b2 at end) ----
                    pc1 = min(c0 + W, SLOTS)
                    if c0 < pc1:
                        gfirst = (c0 + GS - 1) // GS
                        a = gfirst * GS - c0
                        gend = pc1 // GS
                        nfull = gend - gfirst
                        if nfull > 0:
                            nc.vector.tensor_reduce(
                                out=pt[:, gfirst:gend],
                                in_=m2[:, a : a + nfull * GS].rearrange(
                                    "p (g s) -> p g s", s=GS
                                ),
                                axis=mybir.AxisListType.X, op=Alu.max,
                            )
                        if a > 0:  # left partial graph gfirst-1
                            la = min(a, pc1 - c0)
                            tmpm = stat_scr[:, 7:8]
                            nc.vector.tensor_reduce(
                                out=tmpm, in_=m2[:, 0:la],
                                axis=mybir.AxisListType.X, op=Alu.max,
                            )
                            gl = gfirst - 1
                            nc.vector.tensor_tensor(
                                out=pt[:, gl : gl + 1],
                                in0=pt[:, gl : gl + 1], in1=tmpm, op=Alu.max,
                            )
                        r0 = a + max(0, gend - gfirst) * GS
                        if gend >= gfirst and c0 + r0 < pc1:
                            # right partial graph gend (first touch)
                            nc.vector.tensor_reduce(
                                out=pt[:, gend : gend + 1],
                                in_=m2[:, r0 : pc1 - c0],
                                axis=mybir.AxisListType.X, op=Alu.max,
                            )
                    # ---- transpose to node-major for the h table ----
                    if layer < 2:
                        for i in range(W // 128):
                            trp = trpool.tile(
                                [128, 128], dt.bfloat16, name="trp", tag="tr"
                            )
                            nc.tensor.transpose(
                                trp[:], z2[:, i * 128 : (i + 1) * 128],
                                ident[:],
                            )
                            rm = rmpool.tile([128, 128], dt.bfloat16, name="rm")
                            nc.any.tensor_copy(out=rm[:], in_=trp[:])
                            b2i = blo + i
                            kch = 0
                            while cb[kch + 1] <= b2i:
                                kch += 1
                            lr0 = (b2i - cb[kch]) * 128
                            nc.sync.dma_start(
                                out=z_ch[kch][lr0 : lr0 + 128, :], in_=rm[:]
                            )
                        # launch chunk AllGather as soon as its blocks done
                        for kch in range(NCH):
                            if (cb[kch + 1] - 1) // GRP_BLKS == g:
                                agi = nc.gpsimd.collective_compute(
                                    "AllGather", mybir.AluOpType.bypass,
                                    replica_groups=[list(range(C))],
                                    ins=[z_ch[kch].opt()],
                                    outs=[h_ch[layer][kch].opt()],
                                )
                                ag_insts[layer].append(agi)

                # ---- stats reduce + AllReduce ----
                sp = statpool.tile([D, 2], dt.float32, name="sp")
                nc.vector.tensor_reduce(
                    out=sp[:, 0:1], in_=ssum[:, :NG],
                    axis=mybir.AxisListType.X, op=Alu.add,
                )
                nc.vector.tensor_reduce(
                    out=sp[:, 1:2], in_=ssq[:, :NG],
                    axis=mybir.AxisListType.X, op=Alu.add,
                )
                nc.sync.dma_start(out=st_in[layer][:], in_=sp[:])
                nc.gpsimd.collective_compute(
                    "AllReduce", mybir.AluOpType.add,
                    replica_groups=[list(range(C))],
                    ins=[st_in[layer].opt()], outs=[st_out[layer].opt()],
                )
                pass

            # ---- output: affine + transpose + store ----
            compute_fold(2)
            out_big = cpool.tile(
                [128, n_pool_chunks * 3 * D], dt.float32, name="out_big"
            )
            with tc.tile_pool(name="poolt", bufs=2) as ptpool:
                for l in range(3):
                    # pooled z2 = relu(max(m2) + b2); then BN affine
                    pre = ptpool.tile([128, GPC], dt.float32, name="pre")
                    nc.scalar.activation(
                        pre[:], pt_all[l][:], Act.Relu, bias=b2_sb[l][:]
                    )
                    pta = ptpool.tile([128, GPC], dt.float32, name="pta")
                    nc.vector.tensor_scalar(
                        out=pta[:], in0=pre[:],
                        scalar1=s_all[:, l : l + 1],
                        scalar2=t_all[:, l : l + 1], op0=Alu.mult, op1=Alu.add,
                    )
                    for ch in range(n_pool_chunks):
                        rows = (
                            128 if ch < n_pool_chunks - 1 else last_chunk_rows
                        )
                        trp = trpool.tile(
                            [128, 128], dt.float32, name="trpo", tag="tr"
                        )
                        nc.tensor.transpose(
                            trp[:rows, :],
                            pta[:, ch * 128 : ch * 128 + rows], ident32[:],
                        )
                        nc.any.tensor_copy(
                            out=out_big[
                                :rows, ch * 3 * D + l * D : ch * 3 * D
                                + (l + 1) * D
                            ],
                            in_=trp[:rows, :],
                        )
            for ch in range(n_pool_chunks):
                rows = 128 if ch < n_pool_chunks - 1 else last_chunk_rows
                nc.sync.dma_start(
                    out=out_d[ch * 128 : ch * 128 + rows, :],
                    in_=out_big[:rows, ch * 3 * D : (ch + 1) * 3 * D],
                )

    nc.compile()
    for l in range(2):
        base = None
        for k in range(N_CHUNKS):
            mls = nc.lookup_mls(h_ch_handles[l][k])
            addr = mls.memorylocations[0].addr
            shq_k = mls.tensor_shape[0]
            if base is not None:
                assert addr == base, (
                    f"h chunks not contiguous at l={l} k={k}: {addr} != {base}"
                )
            base = addr + shq_k * DIM * 2
    return nc, input_names


def make_in_maps(hd: HostData, inputs: dict, input_names):
    iota = np.tile(np.arange(128, dtype=np.float32), (128, 1)).astype(np_bf16)
    gb = np.zeros((DIM, 6), dtype=np.float32)
    for l in range(3):
        gb[:, 2 * l] = inputs["gamma"][l]
        gb[:, 2 * l + 1] = inputs["beta"][l]
    shared = {
        "x_tbl": hd.x_tbl,
        "iota": np.ascontiguousarray(iota),
        "gb": gb,
    }
    for l in range(3):
        w = np.zeros((DIM, DIM), dtype=np.float32)
        wl = inputs[f"w1_{l}"]
        w[: wl.shape[0], :] = wl
        shared[f"w1_{l}"] = w
        shared[f"w2_{l}"] = np.ascontiguousarray(
            inputs[f"w2_{l}"].astype(np.float32)
        )
        shared[f"b1_{l}"] = inputs[f"b1_{l}"].astype(np.float32).reshape(-1, 1)
        shared[f"b2_{l}"] = inputs[f"b2_{l}"].astype(np.float32).reshape(-1, 1)
    in_maps = []
    for c in range(N_CORES):
        m = dict(shared)
        m["idx"] = hd.idx_sb[c]
        m["rel"] = hd.rel_sb[c]
        m["deg2"] = hd.deg2[c]
        assert set(m.keys()) == set(input_names)
        in_maps.append(m)
    return in_maps


def _run_sharded_timed(nc, in_maps, n_cores, iters=10, warmup=2):
    """Execute the compiled Bass module via PJRT with device-resident inputs,
    timing `iters` back-to-back dispatches (excludes input upload/compile)."""
    import time

    import jax
    from jax.sharding import Mesh, NamedSharding, PartitionSpec
    from jax.experimental.shard_map import shard_map

    import concourse.mybir as mybir
    from concourse import bass2jax

    bass2jax.install_neuronx_cc_hook()
    partition_name = (
        nc.partition_id_tensor.name if nc.partition_id_tensor else None
    )
    in_names, out_names, out_avals, zero_outs = [], [], [], []
    for alloc in nc.m.functions[0].allocations:
        if not isinstance(alloc, mybir.MemoryLocationSet):
            continue
        name = alloc.memorylocations[0].name
        if alloc.kind == "ExternalInput":
            if name != partition_name:
                in_names.append(name)
        elif alloc.kind == "ExternalOutput":
            out_names.append(name)
            shape = tuple(alloc.tensor_shape)
            dtp = mybir.dt.np(alloc.dtype)
            out_avals.append(jax.core.ShapedArray(shape, dtp))
            zero_outs.append(np.zeros(shape, dtp))
    n_params, n_outs = len(in_names), len(out_avals)
    in_names.extend(out_names)
    if partition_name is not None:
        in_names.append(partition_name)
    donate = tuple(range(n_params, n_params + n_outs))

    def _body(*args):
        operands = list(args)
        if partition_name is not None:
            operands.append(bass2jax.partition_id_tensor())
        outs = bass2jax._bass_exec_p.bind(
            *operands,
            out_avals=tuple(out_avals),
            in_names=tuple(in_names),
            out_names=tuple(out_names),
            lowering_input_output_aliases=(),
            sim_require_finite=True,
            sim_require_nnan=True,
            nc=nc,
        )
        return tuple(outs)

    devices = jax.devices()[:n_cores]
    mesh = Mesh(np.asarray(devices), ("core",))
    pspec = PartitionSpec("core")
    in_specs = (pspec,) * (n_params + n_outs)
    sharded = jax.jit(
        shard_map(
            _body, mesh=mesh, in_specs=in_specs,
            out_specs=(pspec,) * len(out_names), check_rep=False,
        ),
        donate_argnums=donate, keep_unused=True,
    )
    shd = NamedSharding(mesh, pspec)
    per_core = [
        [np.asarray(m[name]) for name in in_names[:n_params]] for m in in_maps
    ]
    dev_in = [
        jax.device_put(
            np.concatenate([per_core[c][i] for c in range(n_cores)], axis=0),
            shd,
        )
        for i in range(n_params)
    ]
    n_calls = warmup + (iters if iters else 0)
    zsets = [
        [
            jax.device_put(
                np.zeros((n_cores * z.shape[0], *z.shape[1:]), z.dtype), shd
            )
            for z in zero_outs
        ]
        for _ in range(max(n_calls, 1))
    ]
    outs = None
    for i in range(warmup):
        outs = sharded(*dev_in, *zsets[i])
        jax.block_until_ready(outs)
    dt = None
    if iters:
        t0 = time.perf_counter()
        ress = [sharded(*dev_in, *zsets[warmup + i]) for i in range(iters)]
        jax.block_until_ready(ress)
        dt = (time.perf_counter() - t0) / iters
        outs = ress[-1]
    if outs is None:
        outs = sharded(*dev_in, *zsets[0])
    results = [
        {
            name: np.asarray(outs[i]).reshape(n_cores, *out_avals[i].shape)[c]
            for i, name in enumerate(out_names)
        }
        for c in range(n_cores)
    ]
    return results, dt


def run(inputs: dict, timed: bool = False):
    x = np.asarray(inputs["x"])
    ei = np.asarray(inputs["edge_index"])
    batch = np.asarray(inputs["batch"])
    hd = prep_host(x, ei, batch)
    nc, input_names = build_program(hd)
    in_maps = make_in_maps(hd, inputs, input_names)
    results, dt = _run_sharded_timed(
        nc, in_maps, N_CORES,
        iters=(10 if timed else 0), warmup=(2 if timed else 1),
    )
    outs = [results[c]["pooled"] for c in range(N_CORES)]
    full = np.concatenate(outs, axis=0).astype(np.float32)
    return full, dt


def kernel(**inputs) -> np.ndarray:
    out, _ = run(inputs, timed=False)
    return out
